# revision 31
# baseline (speedup 1.0000x reference)
"""Trainium2 Bass kernel for nn_DDCModel (DDC trajectory filter).

Math (per trajectory b, L sequential steps):
    X_0 = one_hot(init_states[b])                      # [S] distribution
    r_t = X_t . R[a_{b,t}]                             # reward (output)
    X_{t+1} = X_t @ T[a_{b,t}]                         # [S] x [S,S] matvec

Strategy (8 NeuronCores):
  - T is sharded over the output-state axis: core r owns T[:, :, r*512:(r+1)*512]
    cast to fp16 and kept SBUF-resident ([128, A*KT*512] tile layout).
  - Each step every core computes its 512-wide slice of the next interface for
    all 8 trajectories and all 4 actions in one PE pass: the per-action
    stationaries are action-masked copies of X^T ("xhat"), so the PSUM
    accumulation performs the action selection for free. The 4 actions run
    concurrently on disjoint PE column groups (tile_position), quadrupling the
    effective moving-operand bandwidth.
  - The 512-slices are exchanged with a per-step AllGather (fp16, 8KB/rank),
    and one 3D xbar DMA-transpose rebuilds X^T [128, (nt, r*8+b)] in SBUF.
  - Rewards are a second tiny PE pass (moving = R columns, N=1) that
    accumulates into a dedicated PSUM bank (one column per step, evacuated
    once at the end); it also keeps the PE busy during the collective so the
    HAM clock gate stays open. Every core computes the same full rewards,
    so no final gather is needed.

Host-side: actions/init_states are compile-time data - they become the
one-hot mask stream and the initial X^T tile; no dynamic control flow on
device.
"""
import sys

sys.path.insert(0, "/opt/trn_rl_repo")

import numpy as np

N_CORES = 8
B = 8          # trajectories
A = 4          # actions
S = 4096       # state-space size
L = 128        # trajectory length
NS = S // N_CORES       # 512: per-core output-state slice
KT = S // 128           # 32: contraction k-tiles
NT = NS // 128          # 4: per-core n-tiles

_CACHE = {}


def _build(l_steps: int, variant: str = "full", n_repeat: int = 1, n_junk: int = 0):
    from concourse import bass, tile
    from concourse.bass import mybir

    F32 = mybir.dt.float32
    F16 = mybir.dt.float16

    nc = bass.Bass(num_devices=N_CORES)

    t_tiles = nc.declare_dram_parameter("t_tiles", [128, A * KT * NS], F16, isOutput=False)
    r_tiles = nc.declare_dram_parameter("r_tiles", [128, A * KT], F16, isOutput=False)
    x0t = nc.declare_dram_parameter("x0t", [128, NT * 64], F16, isOutput=False)
    masks = nc.declare_dram_parameter("masks", [l_steps, 128, A * NT * 64], F16, isOutput=False)
    out = nc.declare_dram_parameter("out", [B, l_steps], F32, isOutput=True)

    cc_in = [nc.dram_tensor(f"cc_in{i}", [B, NS], F16) for i in range(2)]
    cc_out = [
        nc.dram_tensor(f"cc_out{i}", [N_CORES * B, NS], F16, addr_space="Shared")
        for i in range(2)
    ]
    xabs = nc.dram_tensor("xabs", [1, 8], F16)

    with tile.TileContext(nc) as tc:
        with tc.tile_pool(name="const", bufs=1) as cp, \
             tc.tile_pool(name="loop", bufs=3) as lp, \
             tc.tile_pool(name="ps", bufs=2, space="PSUM") as pmp, \
             tc.tile_pool(name="psj", bufs=2, space="PSUM") as pjp, \
             tc.tile_pool(name="psr", bufs=1, space="PSUM") as prp:

            # ---- resident tensors ----
            t_sb = cp.tile([128, A * KT * NS], F16, tag="t_sb")
            nc.sync.dma_start(out=t_sb[:], in_=t_tiles[:])
            r_sb = cp.tile([128, A * KT], F16, tag="r_sb")
            nc.sync.dma_start(out=r_sb[:], in_=r_tiles[:])
            x0_sb = cp.tile([128, NT * 64], F16, tag="x0_sb")
            nc.sync.dma_start(out=x0_sb[:], in_=x0t[:])

            psum_rew = None
            if variant not in ("ccband", "norew"):
                psum_rew = prp.tile([128, 512], F32, tag="rew")

            for rep in range(n_repeat):
              xt_prev = x0_sb
              for t in range(l_steps):
                  # ---- mask prefetch (replicated across partitions) ----
                  mstep = lp.tile([128, A * NT * 64], F16, tag="mstep")
                  nc.gpsimd.dma_start(out=mstep[:], in_=masks[t])

                  # ---- xhat: action-masked X^T copies ----
                  xhat = lp.tile([128, A * NT * 64], F16, tag="xhat")
                  for a in (range(A) if variant != "ccband" else ()):
                      nc.vector.tensor_tensor(
                          out=xhat[:, a * 256:(a + 1) * 256],
                          in0=xt_prev[:, 0:256],
                          in1=mstep[:, a * 256:(a + 1) * 256],
                          op=mybir.AluOpType.mult,
                      )

                  # ---- main sweep: next-interface slice, 4 actions on 4 PE
                  #      column groups ----
                  pm = pmp.tile([128, NS], F32, tag="pm")
                  if variant != "ccband":
                      for kt in range(KT):
                          for a in range(A):
                              r_, nt_ = kt // NT, kt % NT
                              lhsT = xhat[:, a * 256 + nt_ * 64 + r_ * 8: a * 256 + nt_ * 64 + r_ * 8 + 8]
                              nc.tensor.matmul(
                                  out=pm[32 * a:32 * a + 8, :],
                                  lhsT=lhsT,
                                  rhs=t_sb[:, (a * KT + kt) * NS:(a * KT + kt + 1) * NS],
                                  start=(kt == 0),
                                  stop=(kt == KT - 1),
                                  tile_position=(0, 32 * a),
                              )

                  # ---- reward pass (fills the PE during the collective) ----
                  if variant not in ("norew", "ccband"):
                      for kt in range(KT):
                          for a in range(A):
                              r_, nt_ = kt // NT, kt % NT
                              lhsT = xhat[:, a * 256 + nt_ * 64 + r_ * 8: a * 256 + nt_ * 64 + r_ * 8 + 8]
                              nc.tensor.matmul(
                                  out=psum_rew[32 * a:32 * a + 8, rep * l_steps + t:rep * l_steps + t + 1],
                                  lhsT=lhsT,
                                  rhs=r_sb[:, a * KT + kt:a * KT + kt + 1],
                                  start=(kt == 0),
                                  stop=(kt == KT - 1),
                                  tile_position=(0, 32 * a),
                                  skip_group_check=True,
                              )

                  # ---- junk warmth: keep the PE HAM-warm during the exchange ----
                  if n_junk > 0 and t < l_steps - 1:
                      pj = pjp.tile([128, NS], F32, tag="pj")
                      for j in range(n_junk):
                          nc.tensor.matmul(
                              out=pj[0:8, :],
                              lhsT=xhat[:, 0:8],
                              rhs=t_sb[:, (j % 128) * NS:(j % 128 + 1) * NS],
                              start=True, stop=True,
                              tile_position=(0, 0),
                              skip_group_check=True,
                          )

                  # ---- evacuate + fold the 4 column groups; cast to fp16 ----
                  bounce = lp.tile([B, NS], F16, tag="bounce")
                  if variant == "ccband":
                      nc.vector.tensor_copy(out=bounce[:, 0:256], in_=xt_prev[:8, 0:256])
                  else:
                      c0 = lp.tile([B, NS], F32, tag="c0")
                      nc.vector.tensor_copy(out=c0[:], in_=pm[0:8, :])
                      c1 = lp.tile([B, NS], F32, tag="c1")
                      nc.vector.tensor_add(out=c1[:], in0=c0[:], in1=pm[32:40, :])
                      c2 = lp.tile([B, NS], F32, tag="c2")
                      nc.vector.tensor_add(out=c2[:], in0=c1[:], in1=pm[64:72, :])
                      nc.vector.tensor_add(out=bounce[:], in0=c2[:], in1=pm[96:104, :])

                  if t == l_steps - 1:
                      break

                  if variant == "nocc":
                      continue
                  # ---- exchange the interface slices ----
                  pp = t % 2
                  if variant == "noag":
                      nc.gpsimd.dma_start(out=cc_out[pp][0:8, :], in_=bounce[:])
                  else:
                      nc.gpsimd.dma_start(out=cc_in[pp][:], in_=bounce[:])
                      nc.gpsimd.collective_compute(
                          "AllGather",
                          mybir.AluOpType.bypass,
                          replica_groups=[list(range(N_CORES))],
                          ins=[cc_in[pp][:]],
                          outs=[cc_out[pp][:]],
                      )
                  xt = lp.tile([128, NT * 64], F16, tag="xt")
                  nc.sync.dma_start(
                      out=xt[:].rearrange("p (di m) -> p di m", di=NT),
                      in_=cc_out[pp][:].rearrange("m (di do) -> m di do", do=128),
                      transpose=True,
                  )
                  xt_prev = xt

            # ---- final: fold reward column groups, store ----
            if variant in ("ccband", "norew"):
                zf = cp.tile([B, l_steps], F32, tag="zf")
                nc.vector.memset(zf[:], 0.0)
                nc.gpsimd.dma_start(out=out[:], in_=zf[:])
                raise_skip = True
            else:
                raise_skip = False
            r0 = cp.tile([B, l_steps], F32, tag="r0")
            if not raise_skip:
                nc.vector.tensor_copy(out=r0[:], in_=psum_rew[0:8, 0:l_steps])
            if not raise_skip:
                r1 = cp.tile([B, l_steps], F32, tag="r1")
                nc.vector.tensor_add(out=r1[:], in0=r0[:], in1=psum_rew[32:40, 0:l_steps])
                r2 = cp.tile([B, l_steps], F32, tag="r2")
                nc.vector.tensor_add(out=r2[:], in0=r1[:], in1=psum_rew[64:72, 0:l_steps])
                rfin = cp.tile([B, l_steps], F32, tag="rfin")
                nc.vector.tensor_add(out=rfin[:], in0=r2[:], in1=psum_rew[96:104, 0:l_steps])
                nc.gpsimd.dma_start(out=out[:], in_=rfin[:])

    _split_waits(nc, mybir)
    return nc


def _build_split2(l_steps: int, n_repeat: int = 1):
    """Split the sweep into two 256-col chunks (separate PSUM banks) so each
    chunk's AllGather overlaps the other chunk's matmuls, and order the
    k-tiles so the next sweep starts on chunk-A data while chunk-B's gather
    is still in flight."""
    from concourse import bass, tile
    from concourse.bass import mybir

    F32 = mybir.dt.float32
    F16 = mybir.dt.float16
    HC = NS // 2  # 256: chunk width

    nc = bass.Bass(num_devices=N_CORES)

    t_tiles = nc.declare_dram_parameter("t_tiles", [128, A * KT * NS], F16, isOutput=False)
    r_tiles = nc.declare_dram_parameter("r_tiles", [128, A * KT], F16, isOutput=False)
    x0t = nc.declare_dram_parameter("x0t", [128, NT * 64], F16, isOutput=False)
    masks = nc.declare_dram_parameter("masks", [l_steps, 128, A * NT * 64], F16, isOutput=False)
    out = nc.declare_dram_parameter("out", [B, l_steps], F32, isOutput=True)

    # 2 halves x 2 parities of collective buffers
    cc_in = [nc.dram_tensor(f"cc_in{i}", [B, HC], F16) for i in range(4)]
    cc_out = [
        nc.dram_tensor(f"cc_out{i}", [N_CORES * B, HC], F16, addr_space="Shared")
        for i in range(4)
    ]

    # k-tile order: tiles covered by chunk-A gathers first (nt 0,1)
    kts_af = [kt for kt in range(KT) if kt % NT < 2]
    kts_bf = [kt for kt in range(KT) if kt % NT >= 2]
    kt_order = kts_af + kts_bf

    def lhs_slice(xh, a, kt):
        # xh half-tile layout: [128, a*128 + (nt%2)*64 + r*8]
        r_, nt_ = kt // NT, kt % NT
        c = a * 128 + (nt_ % 2) * 64 + r_ * 8
        return xh[:, c:c + 8]

    with tile.TileContext(nc) as tc:
        with tc.tile_pool(name="const", bufs=1) as cp, \
             tc.tile_pool(name="loop", bufs=3) as lp, \
             tc.tile_pool(name="psA", bufs=2, space="PSUM") as ppa, \
             tc.tile_pool(name="psB", bufs=2, space="PSUM") as ppb, \
             tc.tile_pool(name="psr", bufs=1, space="PSUM") as prp:

            t_sb = cp.tile([128, A * KT * NS], F16, tag="t_sb")
            nc.sync.dma_start(out=t_sb[:], in_=t_tiles[:])
            r_sb = cp.tile([128, A * KT], F16, tag="r_sb")
            nc.sync.dma_start(out=r_sb[:], in_=r_tiles[:])
            x0_sb = cp.tile([128, NT * 64], F16, tag="x0_sb")
            nc.sync.dma_start(out=x0_sb[:], in_=x0t[:])

            psum_rew = prp.tile([128, 512], F32, tag="rew")

            for rep in range(n_repeat):
                xta_prev, xtb_prev = x0_sb[:, 0:128], x0_sb[:, 128:256]
                for t in range(l_steps):
                    mstep = lp.tile([128, A * NT * 64], F16, tag="mstep")
                    nc.gpsimd.dma_start(out=mstep[:], in_=masks[t])

                    # masked X^T halves
                    xha = lp.tile([128, A * 128], F16, tag="xha")
                    xhb = lp.tile([128, A * 128], F16, tag="xhb")
                    for a in range(A):
                        nc.vector.tensor_tensor(
                            out=xha[:, a * 128:(a + 1) * 128],
                            in0=xta_prev,
                            in1=mstep[:, a * 256:a * 256 + 128],
                            op=mybir.AluOpType.mult,
                        )
                    for a in range(A):
                        nc.vector.tensor_tensor(
                            out=xhb[:, a * 128:(a + 1) * 128],
                            in0=xtb_prev,
                            in1=mstep[:, a * 256 + 128:a * 256 + 256],
                            op=mybir.AluOpType.mult,
                        )

                    pmA = ppa.tile([128, HC], F32, tag="pmA")
                    pmB = ppb.tile([128, HC], F32, tag="pmB")
                    last = t == l_steps - 1
                    pp = t % 2

                    for half, pm in ((0, pmA), (1, pmB)):
                        off = half * HC
                        for i, kt in enumerate(kt_order):
                            xh = xha if kt % NT < 2 else xhb
                            for a in range(A):
                                nc.tensor.matmul(
                                    out=pm[32 * a:32 * a + 8, :],
                                    lhsT=lhs_slice(xh, a, kt),
                                    rhs=t_sb[:, (a * KT + kt) * NS + off:
                                             (a * KT + kt) * NS + off + HC],
                                    start=(i == 0),
                                    stop=(i == KT - 1),
                                    tile_position=(0, 32 * a),
                                )
                        # evacuate + fold this chunk, then kick its gather
                        c0 = lp.tile([B, HC], F32, tag=f"c0{half}")
                        nc.vector.tensor_copy(out=c0[:], in_=pm[0:8, :])
                        c1 = lp.tile([B, HC], F32, tag=f"c1{half}")
                        nc.vector.tensor_add(out=c1[:], in0=c0[:], in1=pm[32:40, :])
                        c2 = lp.tile([B, HC], F32, tag=f"c2{half}")
                        nc.vector.tensor_add(out=c2[:], in0=c1[:], in1=pm[64:72, :])
                        bounce = lp.tile([B, HC], F16, tag=f"bounce{half}")
                        nc.vector.tensor_add(out=bounce[:], in0=c2[:], in1=pm[96:104, :])
                        if not last:
                            buf = 2 * pp + half
                            nc.gpsimd.dma_start(out=cc_in[buf][:], in_=bounce[:])
                            nc.gpsimd.collective_compute(
                                "AllGather",
                                mybir.AluOpType.bypass,
                                replica_groups=[list(range(N_CORES))],
                                ins=[cc_in[buf][:]],
                                outs=[cc_out[buf][:]],
                            )

                    # reward pass (overlaps the exchanges)
                    for kt in kt_order:
                        xh = xha if kt % NT < 2 else xhb
                        for a in range(A):
                            nc.tensor.matmul(
                                out=psum_rew[32 * a:32 * a + 8,
                                             rep * l_steps + t:rep * l_steps + t + 1],
                                lhsT=lhs_slice(xh, a, kt),
                                rhs=r_sb[:, a * KT + kt:a * KT + kt + 1],
                                start=(kt == kt_order[0]),
                                stop=(kt == kt_order[-1]),
                                tile_position=(0, 32 * a),
                                skip_group_check=True,
                            )

                    if last:
                        break

                    # transposes: rebuild X^T halves for the next step
                    xta = lp.tile([128, 2 * 64], F16, tag="xta")
                    nc.sync.dma_start(
                        out=xta[:].rearrange("p (di m) -> p di m", di=2),
                        in_=cc_out[2 * pp][:].rearrange("m (di do) -> m di do", do=128),
                        transpose=True,
                    )
                    xtb = lp.tile([128, 2 * 64], F16, tag="xtb")
                    nc.sync.dma_start(
                        out=xtb[:].rearrange("p (di m) -> p di m", di=2),
                        in_=cc_out[2 * pp + 1][:].rearrange("m (di do) -> m di do", do=128),
                        transpose=True,
                    )
                    xta_prev, xtb_prev = xta[:, :], xtb[:, :]

            # final: fold reward column groups, store
            r0 = cp.tile([B, l_steps], F32, tag="r0")
            nc.vector.tensor_copy(out=r0[:], in_=psum_rew[0:8, 0:l_steps])
            r1 = cp.tile([B, l_steps], F32, tag="r1")
            nc.vector.tensor_add(out=r1[:], in0=r0[:], in1=psum_rew[32:40, 0:l_steps])
            r2 = cp.tile([B, l_steps], F32, tag="r2")
            nc.vector.tensor_add(out=r2[:], in0=r1[:], in1=psum_rew[64:72, 0:l_steps])
            rfin = cp.tile([B, l_steps], F32, tag="rfin")
            nc.vector.tensor_add(out=rfin[:], in0=r2[:], in1=psum_rew[96:104, 0:l_steps])
            nc.gpsimd.dma_start(out=out[:], in_=rfin[:])

    _split_waits(nc, mybir)
    return nc


def _build_v2(l_steps: int, n_junk: int = 0):
    """split2 + rewards off the PE.

    Changes vs _build_split2:
      - The reward PE pass (128 tiny N=1 matmuls/step) is replaced by one
        fused DVE tensor_tensor_reduce per half: the per-core partial reward
        r_partial[b] = sum_n X[b,n]*R[a_b,n] over this core's 512-slice is
        accumulated into racc[:, t] in SBUF, and a single final AllReduce
        sums the partials across the 8 cores. r_0 comes from the one-hot
        X_0 slice (x0row) the same way.
      - The PSUM fold drops the tensor_copy (3 adds instead of copy+3 adds).
      - The last transition sweep is skipped entirely: iter t computes
        X_{t+1} whose only uses are rewards r_{t+1} and the next sweep, so
        only l_steps-1 sweeps are needed.
    """
    from concourse import bass, tile
    from concourse.bass import mybir

    F32 = mybir.dt.float32
    F16 = mybir.dt.float16
    HC = NS // 2  # 256: chunk width

    nc = bass.Bass(num_devices=N_CORES)

    t_tiles = nc.declare_dram_parameter("t_tiles", [128, A * KT * NS], F16, isOutput=False)
    x0t = nc.declare_dram_parameter("x0t", [128, NT * 64], F16, isOutput=False)
    x0row = nc.declare_dram_parameter("x0row", [B, NS], F16, isOutput=False)
    masks = nc.declare_dram_parameter("masks", [max(l_steps - 1, 1), 128, A * NT * 64], F16, isOutput=False)
    rsel = nc.declare_dram_parameter("rsel", [l_steps, B, NS], F16, isOutput=False)
    out = nc.declare_dram_parameter("out", [B, l_steps], F32, isOutput=True)

    # 2 halves x 2 parities of collective buffers
    cc_in = [nc.dram_tensor(f"cc_in{i}", [B, HC], F16) for i in range(4)]
    cc_out = [
        nc.dram_tensor(f"cc_out{i}", [N_CORES * B, HC], F16, addr_space="Shared")
        for i in range(4)
    ]
    ar_in = nc.dram_tensor("ar_in", [B, l_steps], F32)
    ar_out = nc.dram_tensor("ar_out", [B, l_steps], F32, addr_space="Shared")

    # k-tile order: tiles covered by chunk-A gathers first (nt 0,1)
    kts_af = [kt for kt in range(KT) if kt % NT < 2]
    kts_bf = [kt for kt in range(KT) if kt % NT >= 2]
    kt_order = kts_af + kts_bf

    def lhs_slice(xh, a, kt):
        # xh half-tile layout: [128, a*128 + (nt%2)*64 + r*8]
        r_, nt_ = kt // NT, kt % NT
        c = a * 128 + (nt_ % 2) * 64 + r_ * 8
        return xh[:, c:c + 8]

    with tile.TileContext(nc) as tc:
        with tc.tile_pool(name="const", bufs=1) as cp, \
             tc.tile_pool(name="loop", bufs=3) as lp, \
             tc.tile_pool(name="psA", bufs=2, space="PSUM") as ppa, \
             tc.tile_pool(name="psB", bufs=2, space="PSUM") as ppb, \
             tc.tile_pool(name="psj", bufs=1, space="PSUM") as pjp:

            t_sb = cp.tile([128, A * KT * NS], F16, tag="t_sb")
            nc.sync.dma_start(out=t_sb[:], in_=t_tiles[:])
            x0_sb = cp.tile([128, NT * 64], F16, tag="x0_sb")
            nc.sync.dma_start(out=x0_sb[:], in_=x0t[:])
            x0r_sb = cp.tile([B, NS], F16, tag="x0r_sb")
            nc.sync.dma_start(out=x0r_sb[:], in_=x0row[:])

            racc = cp.tile([B, l_steps], F32, tag="racc")

            # r_0 partial from the one-hot X_0 slice
            r0sel = lp.tile([B, NS], F16, tag="rselt")
            nc.gpsimd.dma_start(out=r0sel[:], in_=rsel[0])
            prod0 = lp.tile([B, NS], F16, tag="prod")
            nc.vector.tensor_tensor(
                out=prod0[:], in0=x0r_sb[:], in1=r0sel[:],
                op=mybir.AluOpType.mult,
            )
            nc.vector.tensor_reduce(
                out=racc[:, 0:1], in_=prod0[:],
                axis=mybir.AxisListType.X, op=mybir.AluOpType.add,
            )

            xta_prev, xtb_prev = x0_sb[:, 0:128], x0_sb[:, 128:256]
            n_iter = l_steps - 1
            for t in range(n_iter):
                mstep = lp.tile([128, A * NT * 64], F16, tag="mstep")
                nc.gpsimd.dma_start(out=mstep[:], in_=masks[t])
                rselt = lp.tile([B, NS], F16, tag="rselt")
                nc.gpsimd.dma_start(out=rselt[:], in_=rsel[t + 1])

                # masked X^T halves
                xha = lp.tile([128, A * 128], F16, tag="xha")
                xhb = lp.tile([128, A * 128], F16, tag="xhb")
                for a in range(A):
                    nc.vector.tensor_tensor(
                        out=xha[:, a * 128:(a + 1) * 128],
                        in0=xta_prev,
                        in1=mstep[:, a * 256:a * 256 + 128],
                        op=mybir.AluOpType.mult,
                    )
                for a in range(A):
                    nc.vector.tensor_tensor(
                        out=xhb[:, a * 128:(a + 1) * 128],
                        in0=xtb_prev,
                        in1=mstep[:, a * 256 + 128:a * 256 + 256],
                        op=mybir.AluOpType.mult,
                    )

                pmA = ppa.tile([128, HC], F32, tag="pmA")
                pmB = ppb.tile([128, HC], F32, tag="pmB")
                last = t == n_iter - 1
                pp = t % 2

                bounces = []
                for half, pm in ((0, pmA), (1, pmB)):
                    off = half * HC
                    for i, kt in enumerate(kt_order):
                        xh = xha if kt % NT < 2 else xhb
                        for a in range(A):
                            nc.tensor.matmul(
                                out=pm[32 * a:32 * a + 8, :],
                                lhsT=lhs_slice(xh, a, kt),
                                rhs=t_sb[:, (a * KT + kt) * NS + off:
                                         (a * KT + kt) * NS + off + HC],
                                start=(i == 0),
                                stop=(i == KT - 1),
                                tile_position=(0, 32 * a),
                            )
    # fold the 4 column groups: ACT evacuates group 0 (DVE can
                    # read at most one PSUM operand per op), DVE chains 3 adds
                    c0 = lp.tile([B, HC], F32, tag=f"c0{half}")
                    nc.scalar.activation(
                        out=c0[:], in_=pm[0:8, :],
                        func=mybir.ActivationFunctionType.Copy,
                    )
                    c1 = lp.tile([B, HC], F32, tag=f"c1{half}")
                    nc.vector.tensor_tensor(
                        out=c1[:], in0=c0[:], in1=pm[32:40, :],
                        op=mybir.AluOpType.add,
                    )
                    c2 = lp.tile([B, HC], F32, tag=f"c2{half}")
                    nc.vector.tensor_tensor(
                        out=c2[:], in0=c1[:], in1=pm[64:72, :],
                        op=mybir.AluOpType.add,
                    )
                    bounce = lp.tile([B, HC], F16, tag=f"bounce{half}")
                    nc.vector.tensor_tensor(
                        out=bounce[:], in0=c2[:], in1=pm[96:104, :],
                        op=mybir.AluOpType.add,
                    )
                    bounces.append(bounce)
                    if not last:
                        buf = 2 * pp + half
                        nc.sync.dma_start(out=cc_in[buf][:], in_=bounce[:])
                        nc.gpsimd.collective_compute(
                            "AllGather",
                            mybir.AluOpType.bypass,
                            replica_groups=[list(range(N_CORES))],
                            ins=[cc_in[buf][:]],
                            outs=[cc_out[buf][:]],
                        )

                # reward partials r_{t+1} (off the critical path)
                prod = lp.tile([B, NS], F16, tag="prod")
                nc.vector.tensor_tensor(
                    out=prod[:, 0:HC], in0=bounces[0][:], in1=rselt[:, 0:HC],
                    op=mybir.AluOpType.mult,
                )
                nc.vector.tensor_tensor(
                    out=prod[:, HC:NS], in0=bounces[1][:], in1=rselt[:, HC:NS],
                    op=mybir.AluOpType.mult,
                )
                nc.vector.tensor_reduce(
                    out=racc[:, t + 1:t + 2], in_=prod[:],
                    axis=mybir.AxisListType.X, op=mybir.AluOpType.add,
                )

                # HAM warmth: junk matmuls keep the PE clock at 8/8 across
                # the exchange gap (idle >3.4us re-throttles to half rate)
                if n_junk > 0 and not last:
                    pj = pjp.tile([128, HC], F32, tag="pj")
                    for j in range(n_junk):
                        nc.tensor.matmul(
                            out=pj[0:8, :],
                            lhsT=xha[:, 0:8],
                            rhs=t_sb[:, (j % 128) * HC:(j % 128) * HC + HC],
                            start=True, stop=True,
                            tile_position=(0, 0),
                            skip_group_check=True,
                        )

                if last:
                    break

                # transposes: rebuild X^T halves for the next step
                xta = lp.tile([128, 2 * 64], F16, tag="xta")
                nc.sync.dma_start(
                    out=xta[:].rearrange("p (di m) -> p di m", di=2),
                    in_=cc_out[2 * pp][:].rearrange("m (di do) -> m di do", do=128),
                    transpose=True,
                )
                xtb = lp.tile([128, 2 * 64], F16, tag="xtb")
                nc.sync.dma_start(
                    out=xtb[:].rearrange("p (di m) -> p di m", di=2),
                    in_=cc_out[2 * pp + 1][:].rearrange("m (di do) -> m di do", do=128),
                    transpose=True,
                )
                xta_prev, xtb_prev = xta[:, :], xtb[:, :]

            # final: AllReduce the per-core reward partials, store
            nc.sync.dma_start(out=ar_in[:], in_=racc[:])
            nc.gpsimd.collective_compute(
                "AllReduce",
                mybir.AluOpType.add,
                replica_groups=[list(range(N_CORES))],
                ins=[ar_in[:]],
                outs=[ar_out[:]],
            )
            rfin = cp.tile([B, l_steps], F32, tag="rfin")
            nc.sync.dma_start(out=rfin[:], in_=ar_out[:])
            nc.gpsimd.dma_start(out=out[:], in_=rfin[:])

    _split_waits(nc, mybir)
    return nc


def _build_v3(l_steps: int):
    """v2 with the per-step ncfw AllGather replaced by direct core-to-core
    SBUF DMA (remote_dma_broadcast), eliminating the ~6.5us collective floor
    and keeping PE idle gaps under the ~3.4us HAM re-throttle window.

    Layout: each core's next-interface slice [8, 512] is folded into the top
    8 rows of a [16, 512] tile (two [16,256] halves); a HWDGE xbar transpose
    turns each half into [128, 2, 16] (b rows 8..15 are never-read junk), and
    the [128, 32] stage block is broadcast into every core's xt tile at
    column offset 64*rank + 32*half (runtime register offset). xt layout:
    col = 64*r + 32*(nt//2) + 16*(nt%2) + b.

    Arrival sync: 4 semaphores [parity][half]; every broadcast adds 2 per
    dest (16 engines / 8 dests), so after k exchanges of a parity each sem
    reads 16*k. Waits are injected post-scheduling as NoOps on the Vector
    engine right before the first xhat op of each step (Tile's local dep
    chain fold->transpose->bcast->xhat already forbids reorder hazards;
    being one full exchange ahead requires the laggard's own contribution,
    so per-parity cumulative counting is race-free).
    """
    from concourse import bass, tile
    from concourse.bass import mybir

    F32 = mybir.dt.float32
    F16 = mybir.dt.float16
    HC = NS // 2  # 256

    nc = bass.Bass(num_devices=N_CORES)

    t_tiles = nc.declare_dram_parameter("t_tiles", [128, A * KT * NS], F16, isOutput=False)
    x0t = nc.declare_dram_parameter("x0t", [128, NT * 64 * 2], F16, isOutput=False)
    x0row = nc.declare_dram_parameter("x0row", [B, NS], F16, isOutput=False)
    masks = nc.declare_dram_parameter("masks", [max(l_steps - 1, 1), 128, A * 256], F16, isOutput=False)
    rsel = nc.declare_dram_parameter("rsel", [l_steps, B, NS], F16, isOutput=False)
    out = nc.declare_dram_parameter("out", [B, l_steps], F32, isOutput=True)

    bar_buf = nc.dram_tensor("bar_buf", [1, 8], F32)
    bar_out = nc.dram_tensor("bar_out", [1, 8], F32, addr_space="Shared")
    ar_in = nc.dram_tensor("ar_in", [B, l_steps], F32)
    ar_out = nc.dram_tensor("ar_out", [B, l_steps], F32, addr_space="Shared")

    sems = [[nc.alloc_semaphore(f"x_arr_{p}_{h}") for h in range(2)] for p in range(2)]
    lsem = nc.alloc_semaphore("x_sent")

    kts_af = [kt for kt in range(KT) if kt % NT < 2]
    kts_bf = [kt for kt in range(KT) if kt % NT >= 2]
    kt_order = kts_af + kts_bf

    def lhs_slice(xh, a, kt):
        # xhat half layout: [128, a*256 + r*32 + 16*(nt%2) + b]
        r_, nt_ = kt // NT, kt % NT
        c = a * 256 + r_ * 32 + 16 * (nt_ % 2)
        return xh[:, c:c + 8]

    wait_targets = []  # (inst, sem_num, threshold)

    with tile.TileContext(nc) as tc:
        with tc.tile_pool(name="const", bufs=1) as cp, \
             tc.tile_pool(name="loop", bufs=3) as lp, \
             nc.sbuf_tensor("xt_raw", [128, 1024], F16) as xt_raw, \
             tc.tile_pool(name="psA", bufs=2, space="PSUM") as ppa, \
             tc.tile_pool(name="psB", bufs=2, space="PSUM") as ppb:

            t_sb = cp.tile([128, A * KT * NS], F16, tag="t_sb")
            nc.sync.dma_start(out=t_sb[:], in_=t_tiles[:])
            x0_sb = cp.tile([128, NT * 64 * 2], F16, tag="x0_sb")
            nc.sync.dma_start(out=x0_sb[:], in_=x0t[:])
            x0r_sb = cp.tile([B, NS], F16, tag="x0r_sb")
            nc.sync.dma_start(out=x0r_sb[:], in_=x0row[:])
            racc = cp.tile([B, l_steps], F32, tag="racc")

            # startup barrier: no core may send before every core is running
            barsb = cp.tile([1, 8], F32, tag="barsb")
            nc.vector.memset(barsb[:], 0.0)
            nc.sync.dma_start(out=bar_buf[:], in_=barsb[:])
            bar_ci = nc.gpsimd.collective_compute(
                "AllReduce",
                mybir.AluOpType.add,
                replica_groups=[list(range(N_CORES))],
                ins=[bar_buf[:]],
                outs=[bar_out[:]],
            )
            barsb2 = cp.tile([1, 8], F32, tag="barsb2")
            bar_dma = nc.sync.dma_start(out=barsb2[:], in_=bar_out[:])


            # r_0 partial
            r0sel = lp.tile([B, NS], F16, tag="rselt")
            nc.gpsimd.dma_start(out=r0sel[:], in_=rsel[0])
            prod0 = lp.tile([B, NS], F16, tag="prod")
            nc.vector.tensor_tensor(
                out=prod0[:], in0=x0r_sb[:], in1=r0sel[:],
                op=mybir.AluOpType.mult,
            )
            nc.vector.tensor_reduce(
                out=racc[:, 0:1], in_=prod0[:],
                axis=mybir.AxisListType.X, op=mybir.AluOpType.add,
            )

            first_bcasts = []
            n_iter = l_steps - 1
            for t in range(n_iter):
                mstep = lp.tile([128, A * 256], F16, tag="mstep")
                nc.gpsimd.dma_start(out=mstep[:], in_=masks[t])
                rselt = lp.tile([B, NS], F16, tag="rselt")
                nc.gpsimd.dma_start(out=rselt[:], in_=rsel[t + 1])

                # masked X^T halves; [128, 8, 32] views pick each half's
                # 32-col region out of every rank's 64-col block
                xha = lp.tile([128, A * 256], F16, tag="xha")
                xhb = lp.tile([128, A * 256], F16, tag="xhb")
                if t == 0:
                    xt3 = x0_sb[:].rearrange("p (r q) -> p r q", q=64)
                else:
                    base = 512 * ((t - 1) % 2)
                    xt3 = xt_raw[:, base:base + 512].rearrange(
                        "p (r q) -> p r q", q=64)
                first_xh = {}
                for h, xh in ((0, xha), (1, xhb)):
                    for a in range(A):
                        inst = nc.vector.tensor_tensor(
                            out=xh[:, a * 256:(a + 1) * 256].rearrange(
                                "p (r q) -> p r q", q=32),
                            in0=xt3[:, :, 32 * h:32 * h + 32],
                            in1=mstep[:, a * 256:(a + 1) * 256].rearrange(
                                "p (r q) -> p r q", q=32),
                            op=mybir.AluOpType.mult,
                        )
                        if a == 0:
                            first_xh[h] = inst
                if t > 0:
                    par = (t - 1) % 2
                    thr = 16 * ((t - 1) // 2 + 1)
                    wait_targets.append((first_xh[0].ins, sems[par][0].num, thr))
                    wait_targets.append((first_xh[1].ins, sems[par][1].num, thr))

                pmA = ppa.tile([128, HC], F32, tag="pmA")
                pmB = ppb.tile([128, HC], F32, tag="pmB")
                last = t == n_iter - 1
                pp_ = t % 2

                bounces = []
                stages = []
                for half, pm in ((0, pmA), (1, pmB)):
                    off = half * HC
                    for i, kt in enumerate(kt_order):
                        xh = xha if kt % NT < 2 else xhb
                        for a in range(A):
                            nc.tensor.matmul(
                                out=pm[32 * a:32 * a + 8, :],
                                lhsT=lhs_slice(xh, a, kt),
                                rhs=t_sb[:, (a * KT + kt) * NS + off:
                                         (a * KT + kt) * NS + off + HC],
                                start=(i == 0),
                                stop=(i == KT - 1),
                                tile_position=(0, 32 * a),
                            )
                    # fold 4 col groups into rows 0..7 of a 16-row tile
                    c0 = lp.tile([B, HC], F32, tag=f"c0{half}")
                    nc.scalar.activation(
                        out=c0[:], in_=pm[0:8, :],
                        func=mybir.ActivationFunctionType.Copy,
                    )
                    c1 = lp.tile([B, HC], F32, tag=f"c1{half}")
                    nc.vector.tensor_tensor(
                        out=c1[:], in0=c0[:], in1=pm[32:40, :],
                        op=mybir.AluOpType.add,
                    )
                    c2 = lp.tile([B, HC], F32, tag=f"c2{half}")
                    nc.vector.tensor_tensor(
                        out=c2[:], in0=c1[:], in1=pm[64:72, :],
                        op=mybir.AluOpType.add,
                    )
                    b16 = lp.tile([16, HC], F16, tag=f"b16{half}")
                    nc.vector.tensor_tensor(
                        out=b16[0:8, :], in0=c2[:], in1=pm[96:104, :],
                        op=mybir.AluOpType.add,
                    )
                    bounces.append(b16)

                if not last:
                    par = t % 2
                    for half, b16 in ((0, bounces[0]), (1, bounces[1])):
                        stage = lp.tile([128, 32], F16, tag=f"stage{half}")
                        nc.sync.dma_start(
                            out=stage[:].rearrange("p (di m) -> p di m", di=2),
                            in_=b16[:].rearrange("m (di do) -> m di do", do=128),
                            transpose=True,
                        )
                        # slot s of every core holds the block from rank
                        # (self ^ s): one single-dest XOR-relative send per
                        # slot, so every AP is compile-time static.
                        for s_ in range(N_CORES):
                            col = 512 * par + 64 * s_ + 32 * half
                            bc = nc.gpsimd.remote_dma_broadcast(
                                out_ap=xt_raw[:, col:col + 32],
                                in_ap=stage[:],
                                remote_sem=sems[par][half],
                                local_sem=lsem,
                                rdests=[(0, s_) if j == s_ else None
                                        for j in range(N_CORES)],
                            )
                            if t == 0:
                                first_bcasts.append(bc)
                        nc.gpsimd.trigger_dma(count=N_CORES)

                # reward partials r_{t+1}
                prod = lp.tile([B, NS], F16, tag="prod")
                nc.vector.tensor_tensor(
                    out=prod[:, 0:HC], in0=bounces[0][0:8, :], in1=rselt[:, 0:HC],
                    op=mybir.AluOpType.mult,
                )
                nc.vector.tensor_tensor(
                    out=prod[:, HC:NS], in0=bounces[1][0:8, :], in1=rselt[:, HC:NS],
                    op=mybir.AluOpType.mult,
                )
                nc.vector.tensor_reduce(
                    out=racc[:, t + 1:t + 2], in_=prod[:],
                    axis=mybir.AxisListType.X, op=mybir.AluOpType.add,
                )

            # final: AllReduce reward partials, store
            nc.sync.dma_start(out=ar_in[:], in_=racc[:])
            nc.gpsimd.collective_compute(
                "AllReduce",
                mybir.AluOpType.add,
                replica_groups=[list(range(N_CORES))],
                ins=[ar_in[:]],
                outs=[ar_out[:]],
            )
            rfin = cp.tile([B, l_steps], F32, tag="rfin")
            nc.sync.dma_start(out=rfin[:], in_=ar_out[:])
            nc.gpsimd.dma_start(out=out[:], in_=rfin[:])

            for bc in first_bcasts:
                tile.add_dep_helper(
                    bc.ins, bar_dma.ins,
                    reason="no P2P send before the all-core startup barrier",
                )

    _inject_sem_waits(nc, mybir, wait_targets)
    _split_waits(nc, mybir)
    return nc


def _inject_sem_waits(nc, mybir, targets):
    """Insert a NoOp carrying `sem >= thr` immediately before each target
    instruction on its engine (post-scheduling, like _split_waits)."""
    by_inst = {id(inst): (sem_num, thr) for inst, sem_num, thr in targets}
    for bb in nc.main_func.blocks:
        insts = list(bb.instructions)
        new = []
        changed = False
        for ins in insts:
            hit = by_inst.get(id(ins))
            if hit is not None:
                sem_num, thr = hit
                new.append(
                    mybir.InstNoOp(
                        name=f"{ins.name}-xwait",
                        sync_info=mybir.SyncInfo(
                            on_wait=[mybir.SyncWait(
                                sync_type="semaphore",
                                id=sem_num,
                                wait_mode="sem-ge-imm",
                                wait_value=thr,
                            )],
                            on_update=[],
                        ),
                        bass_nofuse=True,
                        engine=ins.engine,
                    )
                )
                changed = True
            new.append(ins)
        if changed:
            live = bb.instructions
            live[:] = new


PROBE_PHASES = [
    (0,), (0, 1), (0, 1, 2), (0, 1, 2, 3),
    (0, 2), (1, 3), (1, 2, 3), (0, 1, 2, 3),
]


def _build_probe2(n_iter: int = 48):
    """Isolate what breaks 4-way col-group concurrency in the real sweep."""
    from concourse import bass, tile
    from concourse.bass import mybir

    F32 = mybir.dt.float32
    F16 = mybir.dt.float16

    nc = bass.Bass(num_devices=N_CORES)
    t_tiles = nc.declare_dram_parameter("t_tiles", [128, A * KT * NS], F16, isOutput=False)
    x0t = nc.declare_dram_parameter("x0t", [128, A * NT * 64], F16, isOutput=False)
    out = nc.declare_dram_parameter("out", [B, 8], F32, isOutput=True)

    CH = 8  # kts per accumulation chain

    with tile.TileContext(nc) as tc:
        with tc.tile_pool(name="const", bufs=1) as cp, \
             tc.tile_pool(name="ps", bufs=2, space="PSUM") as pp, \
             tc.tile_pool(name="psr", bufs=1, space="PSUM") as prp:
            t_sb = cp.tile([128, A * KT * NS], F16, tag="t_sb")
            nc.sync.dma_start(out=t_sb[:], in_=t_tiles[:])
            x_sb = cp.tile([128, A * NT * 64], F16, tag="x_sb")
            nc.sync.dma_start(out=x_sb[:], in_=x0t[:])
            marker = cp.tile([128, 512], F16, tag="marker")
            psum_rew = prp.tile([128, 512], F32, tag="rew")

            def lhs(a, kt):
                r_, nt_ = kt // NT, kt % NT
                c = a * 256 + nt_ * 64 + r_ * 8
                return x_sb[:, c:c + 8]

            # phase a: independent MMs, near rhs slices (probe-1 repro)
            pm = pp.tile([128, NS], F32, tag="pm")
            for it in range(n_iter * CH):
                for g in range(A):
                    nc.tensor.matmul(
                        out=pm[32 * g:32 * g + 8, :],
                        lhsT=lhs(g, it % 4),
                        rhs=t_sb[:, ((it + g) % 4) * NS:((it + g) % 4) * NS + NS],
                        start=True, stop=True,
                        tile_position=(0, 32 * g),
                        skip_group_check=True,
                    )
            nc.vector.memset(marker[:], 1.0)

            # phase b: 8-kt accumulation chains, near rhs slices
            for it in range(n_iter):
                pm = pp.tile([128, NS], F32, tag="pm")
                for kt in range(CH):
                    for g in range(A):
                        nc.tensor.matmul(
                            out=pm[32 * g:32 * g + 8, :],
                            lhsT=lhs(g, kt % 4),
                            rhs=t_sb[:, ((kt + g) % 4) * NS:((kt + g) % 4) * NS + NS],
                            start=(kt == 0), stop=(kt == CH - 1),
                            tile_position=(0, 32 * g),
                        )
            nc.vector.memset(marker[:], 2.0)

            # phase c: chains + REAL far-apart rhs offsets
            for it in range(n_iter):
                pm = pp.tile([128, NS], F32, tag="pm")
                for kt in range(CH):
                    for g in range(A):
                        nc.tensor.matmul(
                            out=pm[32 * g:32 * g + 8, :],
                            lhsT=lhs(g, kt),
                            rhs=t_sb[:, (g * KT + kt) * NS:(g * KT + kt) * NS + NS],
                            start=(kt == 0), stop=(kt == CH - 1),
                            tile_position=(0, 32 * g),
                        )
            nc.vector.memset(marker[:], 3.0)

            # phase d: phase c + reward MMs appended per chain
            for it in range(n_iter):
                pm = pp.tile([128, NS], F32, tag="pm")
                for kt in range(CH):
                    for g in range(A):
                        nc.tensor.matmul(
                            out=pm[32 * g:32 * g + 8, :],
                            lhsT=lhs(g, kt),
                            rhs=t_sb[:, (g * KT + kt) * NS:(g * KT + kt) * NS + NS],
                            start=(kt == 0), stop=(kt == CH - 1),
                            tile_position=(0, 32 * g),
                        )
                for kt in range(CH):
                    for g in range(A):
                        nc.tensor.matmul(
                            out=psum_rew[32 * g:32 * g + 8, it % 512:it % 512 + 1],
                            lhsT=lhs(g, kt),
                            rhs=t_sb[:, (g * KT + kt) * NS:(g * KT + kt) * NS + 1],
                            start=(kt == 0), stop=(kt == CH - 1),
                            tile_position=(0, 32 * g),
                            skip_group_check=True,
                        )
            nc.vector.memset(marker[:], 4.0)

            res = cp.tile([B, 8], F32, tag="res")
            nc.vector.tensor_copy(out=res[:], in_=pm[0:8, 0:8])
            nc.gpsimd.dma_start(out=out[:], in_=res[:])

    _split_waits(nc, mybir)
    return nc


def _build_probe(n_iter: int = 200):
    """Microbench: measure PE column-group concurrency per PROBE_PHASES."""
    from concourse import bass, tile
    from concourse.bass import mybir

    F32 = mybir.dt.float32
    F16 = mybir.dt.float16

    nc = bass.Bass(num_devices=N_CORES)
    t_tiles = nc.declare_dram_parameter("t_tiles", [128, 4 * 512], F16, isOutput=False)
    x0t = nc.declare_dram_parameter("x0t", [128, 64], F16, isOutput=False)
    out = nc.declare_dram_parameter("out", [B, 8], F32, isOutput=True)

    with tile.TileContext(nc) as tc:
        with tc.tile_pool(name="const", bufs=1) as cp, \
             tc.tile_pool(name="ps", bufs=2, space="PSUM") as pp:
            t_sb = cp.tile([128, 4 * 512], F16, tag="t_sb")
            nc.sync.dma_start(out=t_sb[:], in_=t_tiles[:])
            x_sb = cp.tile([128, 64], F16, tag="x_sb")
            nc.sync.dma_start(out=x_sb[:], in_=x0t[:])
            marker = cp.tile([128, 512], F16, tag="marker")

            for phase, groups in enumerate(PROBE_PHASES):
                pm = pp.tile([128, 512], F32, tag="pm")
                for it in range(n_iter):
                    for g in groups:
                        nc.tensor.matmul(
                            out=pm[32 * g:32 * g + 8, :],
                            lhsT=x_sb[:, 8 * g:8 * g + 8],
                            rhs=t_sb[:, 512 * ((it + g) % 4):512 * ((it + g) % 4) + 512],
                            start=True, stop=True,
                            tile_position=(0, 32 * g),
                            skip_group_check=True,
                        )
                # phase marker: big DVE op (visible in trace) + PSUM drain
                nc.vector.memset(marker[:], float(phase))

            res = cp.tile([B, 8], F32, tag="res")
            nc.vector.tensor_copy(out=res[:], in_=pm[0:8, 0:8])
            nc.gpsimd.dma_start(out=out[:], in_=res[:])

    _split_waits(nc, mybir)
    return nc


def _split_waits(nc, mybir, max_waits: int = 1):
    """Walrus rejects >1 sem wait on DMA/CTRL structs; spill extras to NoOps."""
    for bb in nc.main_func.blocks:
        insts = list(bb.instructions)
        new = []
        changed = False
        for ins in insts:
            si = getattr(ins, "sync_info", None)
            if si is not None and len(si.on_wait) > max_waits:
                waits = list(si.on_wait)
                for k, w in enumerate(waits[:-max_waits]):
                    new.append(
                        mybir.InstNoOp(
                            name=f"{ins.name}-wsplit{k}",
                            sync_info=mybir.SyncInfo(on_wait=[w], on_update=[]),
                            bass_nofuse=True,
                            engine=ins.engine,
                        )
                    )
                ins.sync_info = mybir.SyncInfo(
                    on_wait=waits[-max_waits:], on_update=list(si.on_update)
                )
                changed = True
            new.append(ins)
        if changed:
            live = bb.instructions
            live[:] = new


def _prepare_inputs(init_states, actions, T, R, l_steps, variant="full"):
    init_states = np.asarray(init_states).astype(np.int64)
    actions = np.asarray(actions).astype(np.int64)
    T = np.asarray(T, dtype=np.float32)
    R = np.asarray(R, dtype=np.float32)

    T16 = T.astype(np.float16)
    R16 = R.astype(np.float16)

    if variant == "v2":
        # x0row[b, :] = X0[b, r*NS : (r+1)*NS] per core r (one-hot rows)
        # rsel[t, b, :] = R16[actions[b, t], r*NS : (r+1)*NS]
        # masks as in the baseline, truncated to l_steps-1 transitions
        x0t = np.zeros((128, NT, 64), dtype=np.float16)
        for b in range(B):
            s0 = int(init_states[b])
            r_, rem = divmod(s0, NS)
            nt_, p = divmod(rem, 128)
            x0t[p, nt_, r_ * 8 + b] = 1.0
        x0t = x0t.reshape(128, NT * 64)

        n_tr = max(l_steps - 1, 1)
        onehot = (actions.T[:, None, :] == np.arange(A)[None, :, None])  # [L, A, B]
        masks = np.broadcast_to(
            onehot[:, None, :, None, None, :], (actions.shape[1], 128, A, NT, N_CORES, B)
        ).astype(np.float16).reshape(actions.shape[1], 128, A * NT * 64)
        masks = np.ascontiguousarray(masks[:n_tr])

        rsel_full = R16[actions[:, :l_steps]]          # [B, l_steps, S]
        in_maps = []
        for r_ in range(N_CORES):
            tc_ = T16[:, :, r_ * NS:(r_ + 1) * NS]
            tt = np.ascontiguousarray(
                tc_.reshape(A, KT, 128, NS).transpose(2, 0, 1, 3).reshape(128, A * KT * NS)
            )
            x0row = np.zeros((B, NS), dtype=np.float16)
            for b in range(B):
                s0 = int(init_states[b])
                if r_ * NS <= s0 < (r_ + 1) * NS:
                    x0row[b, s0 - r_ * NS] = 1.0
            rsel_c = np.ascontiguousarray(
                rsel_full[:, :, r_ * NS:(r_ + 1) * NS].transpose(1, 0, 2)
            )  # [l_steps, B, NS]
            in_maps.append({
                "t_tiles": tt,
                "x0t": x0t,
                "x0row": x0row,
                "masks": masks,
                "rsel": rsel_c,
            })
        return in_maps

    if variant == "v3":
        # xt col = 64*slot + 32*(nt//2) + 16*(nt%2) + b  (b 8..15 junk);
        # slot s on core d holds the block from rank d^s, so the k-tile
        # kt = s*4+nt of core d's T tile must read input rows of rank d^s.
        n_tr = max(l_steps - 1, 1)
        onehot = (actions.T[:, None, :] == np.arange(A)[None, :, None])  # [L, A, B]
        m6 = np.zeros((actions.shape[1], A, N_CORES, 2, 16), np.float16)
        m6[:, :, :, :, :8] = onehot[:, :, None, None, :]
        masks = np.broadcast_to(
            m6[:, None], (actions.shape[1], 128, A, N_CORES, 2, 16)
        ).reshape(actions.shape[1], 128, A * 256)
        masks = np.ascontiguousarray(masks[:n_tr])

        rsel_full = R16[actions[:, :l_steps]]          # [B, l_steps, S]
        in_maps = []
        for d in range(N_CORES):
            tc_ = T16[:, :, d * NS:(d + 1) * NS]       # [A, S, NS]
            # tt[p, ((a*8 + s)*4 + nt)*NS + n] = T[a, (d^s)*512 + nt*128 + p, n]
            blocks = np.empty((A, N_CORES, NT, 128, NS), np.float16)
            for s in range(N_CORES):
                src = d ^ s
                blocks[:, s] = tc_[:, src * NS:(src + 1) * NS, :].reshape(
                    A, NT, 128, NS)
            tt = np.ascontiguousarray(
                blocks.transpose(3, 0, 1, 2, 4).reshape(128, A * KT * NS))

            x0t = np.zeros((128, N_CORES, 2, 2, 16), dtype=np.float16)
            for b in range(B):
                s0 = int(init_states[b])
                src, rem = divmod(s0, NS)
                nt_, p = divmod(rem, 128)
                x0t[p, d ^ src, nt_ // 2, nt_ % 2, b] = 1.0
            x0t = x0t.reshape(128, N_CORES * 64)

            x0row = np.zeros((B, NS), dtype=np.float16)
            for b in range(B):
                s0 = int(init_states[b])
                if d * NS <= s0 < (d + 1) * NS:
                    x0row[b, s0 - d * NS] = 1.0
            rsel_c = np.ascontiguousarray(
                rsel_full[:, :, d * NS:(d + 1) * NS].transpose(1, 0, 2)
            )
            in_maps.append({
                "t_tiles": tt,
                "x0t": x0t,
                "x0row": x0row,
                "masks": masks,
                "rsel": rsel_c,
            })
        return in_maps

    if variant == "probe":
        tt = np.ascontiguousarray(T16[0, :128 * 1, :].reshape(128, -1)[:, :4 * 512])
        x0t = np.zeros((128, 64), np.float16)
        x0t[:, :] = 0.01
        return [{"t_tiles": tt, "x0t": x0t} for _ in range(N_CORES)]

    if variant == "probe2":
        tc_ = T16[:, :, 0:NS]
        tt = np.ascontiguousarray(
            tc_.reshape(A, KT, 128, NS).transpose(2, 0, 1, 3).reshape(128, A * KT * NS)
        )
        x0t = np.full((128, A * NT * 64), 0.01, np.float16)
        return [{"t_tiles": tt, "x0t": x0t} for _ in range(N_CORES)]

    # r_tiles[p, a*KT + kt] = R[a, kt*128 + p]
    r_tiles = np.ascontiguousarray(
        R16.reshape(A, KT, 128).transpose(2, 0, 1).reshape(128, A * KT)
    )

    # x0t[p, nt*64 + r*8 + b] = X0[b, r*512 + nt*128 + p]
    x0t = np.zeros((128, NT, 64), dtype=np.float16)
    for b in range(B):
        s0 = int(init_states[b])
        r_, rem = divmod(s0, NS)
        nt_, p = divmod(rem, 128)
        x0t[p, nt_, r_ * 8 + b] = 1.0
    x0t = x0t.reshape(128, NT * 64)

    # masks[t, p, a*256 + nt*64 + r*8 + b] = (actions[b, t] == a), all p
    lfull = actions.shape[1]
    onehot = (actions.T[:, None, :] == np.arange(A)[None, :, None])  # [L, A, B]
    masks = np.broadcast_to(
        onehot[:, None, :, None, None, :], (lfull, 128, A, NT, N_CORES, B)
    ).astype(np.float16).reshape(lfull, 128, A * NT * 64)
    masks = np.ascontiguousarray(masks[:l_steps])

    # per-core T tiles: t_tiles[p, (a*KT + kt)*NS + n] = T[a, kt*128+p, r*NS+n]
    in_maps = []
    for r_ in range(N_CORES):
        tc_ = T16[:, :, r_ * NS:(r_ + 1) * NS]                 # [A, S, NS]
        tt = np.ascontiguousarray(
            tc_.reshape(A, KT, 128, NS).transpose(2, 0, 1, 3).reshape(128, A * KT * NS)
        )
        in_maps.append({
            "t_tiles": tt,
            "r_tiles": r_tiles,
            "x0t": x0t,
            "masks": masks,
        })
    return in_maps


def _run(init_states, actions, T, R, l_steps=L, trace=False):
    from concourse.bass_utils import run_bass_kernel_spmd

    import os as _os
    variant = _os.environ.get("KVARIANT", "full")
    n_repeat = int(_os.environ.get("KREPEAT", "1"))
    n_junk = int(_os.environ.get("KJUNK", "0"))
    key = (l_steps, variant, n_repeat, n_junk)
    if key not in _CACHE:
        if variant == "split2":
            _CACHE[key] = _build_split2(l_steps, n_repeat)
        elif variant == "v2":
            _CACHE[key] = _build_v2(l_steps, n_junk)
        elif variant == "probe":
            _CACHE[key] = _build_probe()
        elif variant == "probe2":
            _CACHE[key] = _build_probe2()
        elif variant == "v3":
            _CACHE[key] = _build_v3(l_steps)
        else:
            _CACHE[key] = _build(l_steps, variant, n_repeat, n_junk)
    nc = _CACHE[key]
    in_maps = _prepare_inputs(init_states, actions, T, R, l_steps, variant)
    res = run_bass_kernel_spmd(
        nc, in_maps, list(range(N_CORES)), trace=trace
    )
    rewards = res.results[0]["out"].astype(np.float32)
    return rewards, res


def kernel(init_states, actions, T, R):
    rewards, _ = _run(init_states, actions, T, R, l_steps=L, trace=False)
    return rewards



# revision 32
# speedup vs baseline: 1.2231x; 1.2231x over previous
"""Trainium2 Bass kernel for nn_DDCModel (DDC trajectory filter).

Math (per trajectory b, L sequential steps):
    X_0 = one_hot(init_states[b])                      # [S] distribution
    r_t = X_t . R[a_{b,t}]                             # reward (output)
    X_{t+1} = X_t @ T[a_{b,t}]                         # [S] x [S,S] matvec

Strategy (8 NeuronCores):
  - T is sharded over the output-state axis: core r owns T[:, :, r*512:(r+1)*512]
    cast to fp16 and kept SBUF-resident ([128, A*KT*512] tile layout).
  - Each step every core computes its 512-wide slice of the next interface for
    all 8 trajectories and all 4 actions in one PE pass: the per-action
    stationaries are action-masked copies of X^T ("xhat"), so the PSUM
    accumulation performs the action selection for free. The 4 actions run
    concurrently on disjoint PE column groups (tile_position), quadrupling the
    effective moving-operand bandwidth.
  - The 512-slices are exchanged with a per-step AllGather (fp16, 8KB/rank),
    and one 3D xbar DMA-transpose rebuilds X^T [128, (nt, r*8+b)] in SBUF.
  - Rewards are a second tiny PE pass (moving = R columns, N=1) that
    accumulates into a dedicated PSUM bank (one column per step, evacuated
    once at the end); it also keeps the PE busy during the collective so the
    HAM clock gate stays open. Every core computes the same full rewards,
    so no final gather is needed.

Host-side: actions/init_states are compile-time data - they become the
one-hot mask stream and the initial X^T tile; no dynamic control flow on
device.
"""
import sys

sys.path.insert(0, "/opt/trn_rl_repo")

import numpy as np

N_CORES = 8
B = 8          # trajectories
A = 4          # actions
S = 4096       # state-space size
L = 128        # trajectory length
NS = S // N_CORES       # 512: per-core output-state slice
KT = S // 128           # 32: contraction k-tiles
NT = NS // 128          # 4: per-core n-tiles

_CACHE = {}


def _build(l_steps: int, variant: str = "full", n_repeat: int = 1, n_junk: int = 0):
    from concourse import bass, tile
    from concourse.bass import mybir

    F32 = mybir.dt.float32
    F16 = mybir.dt.float16

    nc = bass.Bass(num_devices=N_CORES)

    t_tiles = nc.declare_dram_parameter("t_tiles", [128, A * KT * NS], F16, isOutput=False)
    r_tiles = nc.declare_dram_parameter("r_tiles", [128, A * KT], F16, isOutput=False)
    x0t = nc.declare_dram_parameter("x0t", [128, NT * 64], F16, isOutput=False)
    masks = nc.declare_dram_parameter("masks", [l_steps, 128, A * NT * 64], F16, isOutput=False)
    out = nc.declare_dram_parameter("out", [B, l_steps], F32, isOutput=True)

    cc_in = [nc.dram_tensor(f"cc_in{i}", [B, NS], F16) for i in range(2)]
    cc_out = [
        nc.dram_tensor(f"cc_out{i}", [N_CORES * B, NS], F16, addr_space="Shared")
        for i in range(2)
    ]
    xabs = nc.dram_tensor("xabs", [1, 8], F16)

    with tile.TileContext(nc) as tc:
        with tc.tile_pool(name="const", bufs=1) as cp, \
             tc.tile_pool(name="loop", bufs=3) as lp, \
             tc.tile_pool(name="ps", bufs=2, space="PSUM") as pmp, \
             tc.tile_pool(name="psj", bufs=2, space="PSUM") as pjp, \
             tc.tile_pool(name="psr", bufs=1, space="PSUM") as prp:

            # ---- resident tensors ----
            t_sb = cp.tile([128, A * KT * NS], F16, tag="t_sb")
            nc.sync.dma_start(out=t_sb[:], in_=t_tiles[:])
            r_sb = cp.tile([128, A * KT], F16, tag="r_sb")
            nc.sync.dma_start(out=r_sb[:], in_=r_tiles[:])
            x0_sb = cp.tile([128, NT * 64], F16, tag="x0_sb")
            nc.sync.dma_start(out=x0_sb[:], in_=x0t[:])

            psum_rew = None
            if variant not in ("ccband", "norew"):
                psum_rew = prp.tile([128, 512], F32, tag="rew")

            for rep in range(n_repeat):
              xt_prev = x0_sb
              for t in range(l_steps):
                  # ---- mask prefetch (replicated across partitions) ----
                  mstep = lp.tile([128, A * NT * 64], F16, tag="mstep")
                  nc.gpsimd.dma_start(out=mstep[:], in_=masks[t])

                  # ---- xhat: action-masked X^T copies ----
                  xhat = lp.tile([128, A * NT * 64], F16, tag="xhat")
                  for a in (range(A) if variant != "ccband" else ()):
                      nc.vector.tensor_tensor(
                          out=xhat[:, a * 256:(a + 1) * 256],
                          in0=xt_prev[:, 0:256],
                          in1=mstep[:, a * 256:(a + 1) * 256],
                          op=mybir.AluOpType.mult,
                      )

                  # ---- main sweep: next-interface slice, 4 actions on 4 PE
                  #      column groups ----
                  pm = pmp.tile([128, NS], F32, tag="pm")
                  if variant != "ccband":
                      for kt in range(KT):
                          for a in range(A):
                              r_, nt_ = kt // NT, kt % NT
                              lhsT = xhat[:, a * 256 + nt_ * 64 + r_ * 8: a * 256 + nt_ * 64 + r_ * 8 + 8]
                              nc.tensor.matmul(
                                  out=pm[32 * a:32 * a + 8, :],
                                  lhsT=lhsT,
                                  rhs=t_sb[:, (a * KT + kt) * NS:(a * KT + kt + 1) * NS],
                                  start=(kt == 0),
                                  stop=(kt == KT - 1),
                                  tile_position=(0, 32 * a),
                              )

                  # ---- reward pass (fills the PE during the collective) ----
                  if variant not in ("norew", "ccband"):
                      for kt in range(KT):
                          for a in range(A):
                              r_, nt_ = kt // NT, kt % NT
                              lhsT = xhat[:, a * 256 + nt_ * 64 + r_ * 8: a * 256 + nt_ * 64 + r_ * 8 + 8]
                              nc.tensor.matmul(
                                  out=psum_rew[32 * a:32 * a + 8, rep * l_steps + t:rep * l_steps + t + 1],
                                  lhsT=lhsT,
                                  rhs=r_sb[:, a * KT + kt:a * KT + kt + 1],
                                  start=(kt == 0),
                                  stop=(kt == KT - 1),
                                  tile_position=(0, 32 * a),
                                  skip_group_check=True,
                              )

                  # ---- junk warmth: keep the PE HAM-warm during the exchange ----
                  if n_junk > 0 and t < l_steps - 1:
                      pj = pjp.tile([128, NS], F32, tag="pj")
                      for j in range(n_junk):
                          nc.tensor.matmul(
                              out=pj[0:8, :],
                              lhsT=xhat[:, 0:8],
                              rhs=t_sb[:, (j % 128) * NS:(j % 128 + 1) * NS],
                              start=True, stop=True,
                              tile_position=(0, 0),
                              skip_group_check=True,
                          )

                  # ---- evacuate + fold the 4 column groups; cast to fp16 ----
                  bounce = lp.tile([B, NS], F16, tag="bounce")
                  if variant == "ccband":
                      nc.vector.tensor_copy(out=bounce[:, 0:256], in_=xt_prev[:8, 0:256])
                  else:
                      c0 = lp.tile([B, NS], F32, tag="c0")
                      nc.vector.tensor_copy(out=c0[:], in_=pm[0:8, :])
                      c1 = lp.tile([B, NS], F32, tag="c1")
                      nc.vector.tensor_add(out=c1[:], in0=c0[:], in1=pm[32:40, :])
                      c2 = lp.tile([B, NS], F32, tag="c2")
                      nc.vector.tensor_add(out=c2[:], in0=c1[:], in1=pm[64:72, :])
                      nc.vector.tensor_add(out=bounce[:], in0=c2[:], in1=pm[96:104, :])

                  if t == l_steps - 1:
                      break

                  if variant == "nocc":
                      continue
                  # ---- exchange the interface slices ----
                  pp = t % 2
                  if variant == "noag":
                      nc.gpsimd.dma_start(out=cc_out[pp][0:8, :], in_=bounce[:])
                  else:
                      nc.gpsimd.dma_start(out=cc_in[pp][:], in_=bounce[:])
                      nc.gpsimd.collective_compute(
                          "AllGather",
                          mybir.AluOpType.bypass,
                          replica_groups=[list(range(N_CORES))],
                          ins=[cc_in[pp][:]],
                          outs=[cc_out[pp][:]],
                      )
                  xt = lp.tile([128, NT * 64], F16, tag="xt")
                  nc.sync.dma_start(
                      out=xt[:].rearrange("p (di m) -> p di m", di=NT),
                      in_=cc_out[pp][:].rearrange("m (di do) -> m di do", do=128),
                      transpose=True,
                  )
                  xt_prev = xt

            # ---- final: fold reward column groups, store ----
            if variant in ("ccband", "norew"):
                zf = cp.tile([B, l_steps], F32, tag="zf")
                nc.vector.memset(zf[:], 0.0)
                nc.gpsimd.dma_start(out=out[:], in_=zf[:])
                raise_skip = True
            else:
                raise_skip = False
            r0 = cp.tile([B, l_steps], F32, tag="r0")
            if not raise_skip:
                nc.vector.tensor_copy(out=r0[:], in_=psum_rew[0:8, 0:l_steps])
            if not raise_skip:
                r1 = cp.tile([B, l_steps], F32, tag="r1")
                nc.vector.tensor_add(out=r1[:], in0=r0[:], in1=psum_rew[32:40, 0:l_steps])
                r2 = cp.tile([B, l_steps], F32, tag="r2")
                nc.vector.tensor_add(out=r2[:], in0=r1[:], in1=psum_rew[64:72, 0:l_steps])
                rfin = cp.tile([B, l_steps], F32, tag="rfin")
                nc.vector.tensor_add(out=rfin[:], in0=r2[:], in1=psum_rew[96:104, 0:l_steps])
                nc.gpsimd.dma_start(out=out[:], in_=rfin[:])

    _split_waits(nc, mybir)
    return nc


def _build_split2(l_steps: int, n_repeat: int = 1):
    """Split the sweep into two 256-col chunks (separate PSUM banks) so each
    chunk's AllGather overlaps the other chunk's matmuls, and order the
    k-tiles so the next sweep starts on chunk-A data while chunk-B's gather
    is still in flight."""
    from concourse import bass, tile
    from concourse.bass import mybir

    F32 = mybir.dt.float32
    F16 = mybir.dt.float16
    HC = NS // 2  # 256: chunk width

    nc = bass.Bass(num_devices=N_CORES)

    t_tiles = nc.declare_dram_parameter("t_tiles", [128, A * KT * NS], F16, isOutput=False)
    r_tiles = nc.declare_dram_parameter("r_tiles", [128, A * KT], F16, isOutput=False)
    x0t = nc.declare_dram_parameter("x0t", [128, NT * 64], F16, isOutput=False)
    masks = nc.declare_dram_parameter("masks", [l_steps, 128, A * NT * 64], F16, isOutput=False)
    out = nc.declare_dram_parameter("out", [B, l_steps], F32, isOutput=True)

    # 2 halves x 2 parities of collective buffers
    cc_in = [nc.dram_tensor(f"cc_in{i}", [B, HC], F16) for i in range(4)]
    cc_out = [
        nc.dram_tensor(f"cc_out{i}", [N_CORES * B, HC], F16, addr_space="Shared")
        for i in range(4)
    ]

    # k-tile order: tiles covered by chunk-A gathers first (nt 0,1)
    kts_af = [kt for kt in range(KT) if kt % NT < 2]
    kts_bf = [kt for kt in range(KT) if kt % NT >= 2]
    kt_order = kts_af + kts_bf

    def lhs_slice(xh, a, kt):
        # xh half-tile layout: [128, a*128 + (nt%2)*64 + r*8]
        r_, nt_ = kt // NT, kt % NT
        c = a * 128 + (nt_ % 2) * 64 + r_ * 8
        return xh[:, c:c + 8]

    with tile.TileContext(nc) as tc:
        with tc.tile_pool(name="const", bufs=1) as cp, \
             tc.tile_pool(name="loop", bufs=3) as lp, \
             tc.tile_pool(name="psA", bufs=2, space="PSUM") as ppa, \
             tc.tile_pool(name="psB", bufs=2, space="PSUM") as ppb, \
             tc.tile_pool(name="psr", bufs=1, space="PSUM") as prp:

            t_sb = cp.tile([128, A * KT * NS], F16, tag="t_sb")
            nc.sync.dma_start(out=t_sb[:], in_=t_tiles[:])
            r_sb = cp.tile([128, A * KT], F16, tag="r_sb")
            nc.sync.dma_start(out=r_sb[:], in_=r_tiles[:])
            x0_sb = cp.tile([128, NT * 64], F16, tag="x0_sb")
            nc.sync.dma_start(out=x0_sb[:], in_=x0t[:])

            psum_rew = prp.tile([128, 512], F32, tag="rew")

            for rep in range(n_repeat):
                xta_prev, xtb_prev = x0_sb[:, 0:128], x0_sb[:, 128:256]
                for t in range(l_steps):
                    mstep = lp.tile([128, A * NT * 64], F16, tag="mstep")
                    nc.gpsimd.dma_start(out=mstep[:], in_=masks[t])

                    # masked X^T halves
                    xha = lp.tile([128, A * 128], F16, tag="xha")
                    xhb = lp.tile([128, A * 128], F16, tag="xhb")
                    for a in range(A):
                        nc.vector.tensor_tensor(
                            out=xha[:, a * 128:(a + 1) * 128],
                            in0=xta_prev,
                            in1=mstep[:, a * 256:a * 256 + 128],
                            op=mybir.AluOpType.mult,
                        )
                    for a in range(A):
                        nc.vector.tensor_tensor(
                            out=xhb[:, a * 128:(a + 1) * 128],
                            in0=xtb_prev,
                            in1=mstep[:, a * 256 + 128:a * 256 + 256],
                            op=mybir.AluOpType.mult,
                        )

                    pmA = ppa.tile([128, HC], F32, tag="pmA")
                    pmB = ppb.tile([128, HC], F32, tag="pmB")
                    last = t == l_steps - 1
                    pp = t % 2

                    for half, pm in ((0, pmA), (1, pmB)):
                        off = half * HC
                        for i, kt in enumerate(kt_order):
                            xh = xha if kt % NT < 2 else xhb
                            for a in range(A):
                                nc.tensor.matmul(
                                    out=pm[32 * a:32 * a + 8, :],
                                    lhsT=lhs_slice(xh, a, kt),
                                    rhs=t_sb[:, (a * KT + kt) * NS + off:
                                             (a * KT + kt) * NS + off + HC],
                                    start=(i == 0),
                                    stop=(i == KT - 1),
                                    tile_position=(0, 32 * a),
                                )
                        # evacuate + fold this chunk, then kick its gather
                        c0 = lp.tile([B, HC], F32, tag=f"c0{half}")
                        nc.vector.tensor_copy(out=c0[:], in_=pm[0:8, :])
                        c1 = lp.tile([B, HC], F32, tag=f"c1{half}")
                        nc.vector.tensor_add(out=c1[:], in0=c0[:], in1=pm[32:40, :])
                        c2 = lp.tile([B, HC], F32, tag=f"c2{half}")
                        nc.vector.tensor_add(out=c2[:], in0=c1[:], in1=pm[64:72, :])
                        bounce = lp.tile([B, HC], F16, tag=f"bounce{half}")
                        nc.vector.tensor_add(out=bounce[:], in0=c2[:], in1=pm[96:104, :])
                        if not last:
                            buf = 2 * pp + half
                            nc.gpsimd.dma_start(out=cc_in[buf][:], in_=bounce[:])
                            nc.gpsimd.collective_compute(
                                "AllGather",
                                mybir.AluOpType.bypass,
                                replica_groups=[list(range(N_CORES))],
                                ins=[cc_in[buf][:]],
                                outs=[cc_out[buf][:]],
                            )

                    # reward pass (overlaps the exchanges)
                    for kt in kt_order:
                        xh = xha if kt % NT < 2 else xhb
                        for a in range(A):
                            nc.tensor.matmul(
                                out=psum_rew[32 * a:32 * a + 8,
                                             rep * l_steps + t:rep * l_steps + t + 1],
                                lhsT=lhs_slice(xh, a, kt),
                                rhs=r_sb[:, a * KT + kt:a * KT + kt + 1],
                                start=(kt == kt_order[0]),
                                stop=(kt == kt_order[-1]),
                                tile_position=(0, 32 * a),
                                skip_group_check=True,
                            )

                    if last:
                        break

                    # transposes: rebuild X^T halves for the next step
                    xta = lp.tile([128, 2 * 64], F16, tag="xta")
                    nc.sync.dma_start(
                        out=xta[:].rearrange("p (di m) -> p di m", di=2),
                        in_=cc_out[2 * pp][:].rearrange("m (di do) -> m di do", do=128),
                        transpose=True,
                    )
                    xtb = lp.tile([128, 2 * 64], F16, tag="xtb")
                    nc.sync.dma_start(
                        out=xtb[:].rearrange("p (di m) -> p di m", di=2),
                        in_=cc_out[2 * pp + 1][:].rearrange("m (di do) -> m di do", do=128),
                        transpose=True,
                    )
                    xta_prev, xtb_prev = xta[:, :], xtb[:, :]

            # final: fold reward column groups, store
            r0 = cp.tile([B, l_steps], F32, tag="r0")
            nc.vector.tensor_copy(out=r0[:], in_=psum_rew[0:8, 0:l_steps])
            r1 = cp.tile([B, l_steps], F32, tag="r1")
            nc.vector.tensor_add(out=r1[:], in0=r0[:], in1=psum_rew[32:40, 0:l_steps])
            r2 = cp.tile([B, l_steps], F32, tag="r2")
            nc.vector.tensor_add(out=r2[:], in0=r1[:], in1=psum_rew[64:72, 0:l_steps])
            rfin = cp.tile([B, l_steps], F32, tag="rfin")
            nc.vector.tensor_add(out=rfin[:], in0=r2[:], in1=psum_rew[96:104, 0:l_steps])
            nc.gpsimd.dma_start(out=out[:], in_=rfin[:])

    _split_waits(nc, mybir)
    return nc


def _build_v2(l_steps: int, n_junk: int = 0):
    """split2 + rewards off the PE.

    Changes vs _build_split2:
      - The reward PE pass (128 tiny N=1 matmuls/step) is replaced by one
        fused DVE tensor_tensor_reduce per half: the per-core partial reward
        r_partial[b] = sum_n X[b,n]*R[a_b,n] over this core's 512-slice is
        accumulated into racc[:, t] in SBUF, and a single final AllReduce
        sums the partials across the 8 cores. r_0 comes from the one-hot
        X_0 slice (x0row) the same way.
      - The PSUM fold drops the tensor_copy (3 adds instead of copy+3 adds).
      - The last transition sweep is skipped entirely: iter t computes
        X_{t+1} whose only uses are rewards r_{t+1} and the next sweep, so
        only l_steps-1 sweeps are needed.
    """
    from concourse import bass, tile
    from concourse.bass import mybir

    F32 = mybir.dt.float32
    F16 = mybir.dt.float16
    HC = NS // 2  # 256: chunk width

    nc = bass.Bass(num_devices=N_CORES)

    t_tiles = nc.declare_dram_parameter("t_tiles", [128, A * KT * NS], F16, isOutput=False)
    x0t = nc.declare_dram_parameter("x0t", [128, NT * 64], F16, isOutput=False)
    x0row = nc.declare_dram_parameter("x0row", [B, NS], F16, isOutput=False)
    masks = nc.declare_dram_parameter("masks", [max(l_steps - 1, 1), 128, A * NT * 64], F16, isOutput=False)
    rsel = nc.declare_dram_parameter("rsel", [l_steps, B, NS], F16, isOutput=False)
    out = nc.declare_dram_parameter("out", [B, l_steps], F32, isOutput=True)

    # 2 halves x 2 parities of collective buffers
    cc_in = [nc.dram_tensor(f"cc_in{i}", [B, HC], F16) for i in range(4)]
    cc_out = [
        nc.dram_tensor(f"cc_out{i}", [N_CORES * B, HC], F16, addr_space="Shared")
        for i in range(4)
    ]
    ar_in = nc.dram_tensor("ar_in", [B, l_steps], F32)
    ar_out = nc.dram_tensor("ar_out", [B, l_steps], F32, addr_space="Shared")

    # k-tile order: tiles covered by chunk-A gathers first (nt 0,1)
    kts_af = [kt for kt in range(KT) if kt % NT < 2]
    kts_bf = [kt for kt in range(KT) if kt % NT >= 2]
    kt_order = kts_af + kts_bf

    def lhs_slice(xh, a, kt):
        # xh half-tile layout: [128, a*128 + (nt%2)*64 + r*8]
        r_, nt_ = kt // NT, kt % NT
        c = a * 128 + (nt_ % 2) * 64 + r_ * 8
        return xh[:, c:c + 8]

    with tile.TileContext(nc) as tc:
        with tc.tile_pool(name="const", bufs=1) as cp, \
             tc.tile_pool(name="loop", bufs=3) as lp, \
             tc.tile_pool(name="psA", bufs=2, space="PSUM") as ppa, \
             tc.tile_pool(name="psB", bufs=2, space="PSUM") as ppb, \
             tc.tile_pool(name="psj", bufs=1, space="PSUM") as pjp:

            t_sb = cp.tile([128, A * KT * NS], F16, tag="t_sb")
            nc.sync.dma_start(out=t_sb[:], in_=t_tiles[:])
            x0_sb = cp.tile([128, NT * 64], F16, tag="x0_sb")
            nc.sync.dma_start(out=x0_sb[:], in_=x0t[:])
            x0r_sb = cp.tile([B, NS], F16, tag="x0r_sb")
            nc.sync.dma_start(out=x0r_sb[:], in_=x0row[:])

            racc = cp.tile([B, l_steps], F32, tag="racc")

            # r_0 partial from the one-hot X_0 slice
            r0sel = lp.tile([B, NS], F16, tag="rselt")
            nc.gpsimd.dma_start(out=r0sel[:], in_=rsel[0])
            prod0 = lp.tile([B, NS], F16, tag="prod")
            nc.vector.tensor_tensor(
                out=prod0[:], in0=x0r_sb[:], in1=r0sel[:],
                op=mybir.AluOpType.mult,
            )
            nc.vector.tensor_reduce(
                out=racc[:, 0:1], in_=prod0[:],
                axis=mybir.AxisListType.X, op=mybir.AluOpType.add,
            )

            xta_prev, xtb_prev = x0_sb[:, 0:128], x0_sb[:, 128:256]
            n_iter = l_steps - 1
            for t in range(n_iter):
                mstep = lp.tile([128, A * NT * 64], F16, tag="mstep")
                nc.gpsimd.dma_start(out=mstep[:], in_=masks[t])
                rselt = lp.tile([B, NS], F16, tag="rselt")
                nc.gpsimd.dma_start(out=rselt[:], in_=rsel[t + 1])

                # masked X^T halves
                xha = lp.tile([128, A * 128], F16, tag="xha")
                xhb = lp.tile([128, A * 128], F16, tag="xhb")
                for a in range(A):
                    nc.vector.tensor_tensor(
                        out=xha[:, a * 128:(a + 1) * 128],
                        in0=xta_prev,
                        in1=mstep[:, a * 256:a * 256 + 128],
                        op=mybir.AluOpType.mult,
                    )
                for a in range(A):
                    nc.vector.tensor_tensor(
                        out=xhb[:, a * 128:(a + 1) * 128],
                        in0=xtb_prev,
                        in1=mstep[:, a * 256 + 128:a * 256 + 256],
                        op=mybir.AluOpType.mult,
                    )

                pmA = ppa.tile([128, HC], F32, tag="pmA")
                pmB = ppb.tile([128, HC], F32, tag="pmB")
                last = t == n_iter - 1
                pp = t % 2

                bounces = []
                for half, pm in ((0, pmA), (1, pmB)):
                    off = half * HC
                    for i, kt in enumerate(kt_order):
                        xh = xha if kt % NT < 2 else xhb
                        for a in range(A):
                            nc.tensor.matmul(
                                out=pm[32 * a:32 * a + 8, :],
                                lhsT=lhs_slice(xh, a, kt),
                                rhs=t_sb[:, (a * KT + kt) * NS + off:
                                         (a * KT + kt) * NS + off + HC],
                                start=(i == 0),
                                stop=(i == KT - 1),
                                tile_position=(0, 32 * a),
                            )
    # fold the 4 column groups: ACT evacuates group 0 (DVE can
                    # read at most one PSUM operand per op), DVE chains 3 adds
                    c0 = lp.tile([B, HC], F32, tag=f"c0{half}")
                    nc.scalar.activation(
                        out=c0[:], in_=pm[0:8, :],
                        func=mybir.ActivationFunctionType.Copy,
                    )
                    c1 = lp.tile([B, HC], F32, tag=f"c1{half}")
                    nc.vector.tensor_tensor(
                        out=c1[:], in0=c0[:], in1=pm[32:40, :],
                        op=mybir.AluOpType.add,
                    )
                    c2 = lp.tile([B, HC], F32, tag=f"c2{half}")
                    nc.vector.tensor_tensor(
                        out=c2[:], in0=c1[:], in1=pm[64:72, :],
                        op=mybir.AluOpType.add,
                    )
                    bounce = lp.tile([B, HC], F16, tag=f"bounce{half}")
                    nc.vector.tensor_tensor(
                        out=bounce[:], in0=c2[:], in1=pm[96:104, :],
                        op=mybir.AluOpType.add,
                    )
                    bounces.append(bounce)
                    if not last:
                        buf = 2 * pp + half
                        nc.sync.dma_start(out=cc_in[buf][:], in_=bounce[:])
                        nc.gpsimd.collective_compute(
                            "AllGather",
                            mybir.AluOpType.bypass,
                            replica_groups=[list(range(N_CORES))],
                            ins=[cc_in[buf][:]],
                            outs=[cc_out[buf][:]],
                        )

                # reward partials r_{t+1} (off the critical path)
                prod = lp.tile([B, NS], F16, tag="prod")
                nc.vector.tensor_tensor(
                    out=prod[:, 0:HC], in0=bounces[0][:], in1=rselt[:, 0:HC],
                    op=mybir.AluOpType.mult,
                )
                nc.vector.tensor_tensor(
                    out=prod[:, HC:NS], in0=bounces[1][:], in1=rselt[:, HC:NS],
                    op=mybir.AluOpType.mult,
                )
                nc.vector.tensor_reduce(
                    out=racc[:, t + 1:t + 2], in_=prod[:],
                    axis=mybir.AxisListType.X, op=mybir.AluOpType.add,
                )

                # HAM warmth: junk matmuls keep the PE clock at 8/8 across
                # the exchange gap (idle >3.4us re-throttles to half rate)
                if n_junk > 0 and not last:
                    pj = pjp.tile([128, HC], F32, tag="pj")
                    for j in range(n_junk):
                        nc.tensor.matmul(
                            out=pj[0:8, :],
                            lhsT=xha[:, 0:8],
                            rhs=t_sb[:, (j % 128) * HC:(j % 128) * HC + HC],
                            start=True, stop=True,
                            tile_position=(0, 0),
                            skip_group_check=True,
                        )

                if last:
                    break

                # transposes: rebuild X^T halves for the next step
                xta = lp.tile([128, 2 * 64], F16, tag="xta")
                nc.sync.dma_start(
                    out=xta[:].rearrange("p (di m) -> p di m", di=2),
                    in_=cc_out[2 * pp][:].rearrange("m (di do) -> m di do", do=128),
                    transpose=True,
                )
                xtb = lp.tile([128, 2 * 64], F16, tag="xtb")
                nc.sync.dma_start(
                    out=xtb[:].rearrange("p (di m) -> p di m", di=2),
                    in_=cc_out[2 * pp + 1][:].rearrange("m (di do) -> m di do", do=128),
                    transpose=True,
                )
                xta_prev, xtb_prev = xta[:, :], xtb[:, :]

            # final: AllReduce the per-core reward partials, store
            nc.sync.dma_start(out=ar_in[:], in_=racc[:])
            nc.gpsimd.collective_compute(
                "AllReduce",
                mybir.AluOpType.add,
                replica_groups=[list(range(N_CORES))],
                ins=[ar_in[:]],
                outs=[ar_out[:]],
            )
            rfin = cp.tile([B, l_steps], F32, tag="rfin")
            nc.sync.dma_start(out=rfin[:], in_=ar_out[:])
            nc.gpsimd.dma_start(out=out[:], in_=rfin[:])

    _split_waits(nc, mybir)
    return nc


def _build_v3(l_steps: int):
    """v2 with the per-step ncfw AllGather replaced by direct core-to-core
    SBUF DMA (remote_dma_broadcast), eliminating the ~6.5us collective floor
    and keeping PE idle gaps under the ~3.4us HAM re-throttle window.

    Layout: each core's next-interface slice [8, 512] is folded into the top
    8 rows of a [16, 512] tile (two [16,256] halves); a HWDGE xbar transpose
    turns each half into [128, 2, 16] (b rows 8..15 are never-read junk), and
    the [128, 32] stage block is broadcast into every core's xt tile at
    column offset 64*rank + 32*half (runtime register offset). xt layout:
    col = 64*r + 32*(nt//2) + 16*(nt%2) + b.

    Arrival sync: 4 semaphores [parity][half]; every broadcast adds 2 per
    dest (16 engines / 8 dests), so after k exchanges of a parity each sem
    reads 16*k. Waits are injected post-scheduling as NoOps on the Vector
    engine right before the first xhat op of each step (Tile's local dep
    chain fold->transpose->bcast->xhat already forbids reorder hazards;
    being one full exchange ahead requires the laggard's own contribution,
    so per-parity cumulative counting is race-free).
    """
    from concourse import bass, tile
    from concourse.bass import mybir

    F32 = mybir.dt.float32
    F16 = mybir.dt.float16
    HC = NS // 2  # 256

    nc = bass.Bass(num_devices=N_CORES)

    t_tiles = nc.declare_dram_parameter("t_tiles", [128, A * KT * NS], F16, isOutput=False)
    x0t = nc.declare_dram_parameter("x0t", [128, NT * 64 * 2], F16, isOutput=False)
    x0row = nc.declare_dram_parameter("x0row", [B, NS], F16, isOutput=False)
    masks = nc.declare_dram_parameter("masks", [max(l_steps - 1, 1), 128, A * 256], F16, isOutput=False)
    rsel = nc.declare_dram_parameter("rsel", [l_steps, B, NS], F16, isOutput=False)
    out = nc.declare_dram_parameter("out", [B, l_steps], F32, isOutput=True)

    bar_buf = nc.dram_tensor("bar_buf", [1, 8], F32)
    bar_out = nc.dram_tensor("bar_out", [1, 8], F32, addr_space="Shared")
    ar_in = nc.dram_tensor("ar_in", [B, l_steps], F32)
    ar_out = nc.dram_tensor("ar_out", [B, l_steps], F32, addr_space="Shared")

    sems = [[nc.alloc_semaphore(f"x_arr_{p}_{h}") for h in range(2)] for p in range(2)]
    lsem = nc.alloc_semaphore("x_sent")

    kts_af = [kt for kt in range(KT) if kt % NT < 2]
    kts_bf = [kt for kt in range(KT) if kt % NT >= 2]
    kt_order = kts_af + kts_bf

    def lhs_slice(xh, a, kt):
        # xhat half layout: [128, a*256 + r*32 + 16*(nt%2) + b]
        r_, nt_ = kt // NT, kt % NT
        c = a * 256 + r_ * 32 + 16 * (nt_ % 2)
        return xh[:, c:c + 8]

    wait_targets = []  # (inst, sem_num, threshold)

    with tile.TileContext(nc) as tc:
        with tc.tile_pool(name="const", bufs=1) as cp, \
             tc.tile_pool(name="loop", bufs=3) as lp, \
             nc.sbuf_tensor("xt_raw", [128, 1024], F16) as xt_raw, \
             tc.tile_pool(name="psA", bufs=2, space="PSUM") as ppa, \
             tc.tile_pool(name="psB", bufs=2, space="PSUM") as ppb:

            t_sb = cp.tile([128, A * KT * NS], F16, tag="t_sb")
            nc.sync.dma_start(out=t_sb[:], in_=t_tiles[:])
            x0_sb = cp.tile([128, NT * 64 * 2], F16, tag="x0_sb")
            nc.sync.dma_start(out=x0_sb[:], in_=x0t[:])
            x0r_sb = cp.tile([B, NS], F16, tag="x0r_sb")
            nc.sync.dma_start(out=x0r_sb[:], in_=x0row[:])
            racc = cp.tile([B, l_steps], F32, tag="racc")

            # startup barrier: no core may send before every core is running
            barsb = cp.tile([1, 8], F32, tag="barsb")
            nc.vector.memset(barsb[:], 0.0)
            nc.sync.dma_start(out=bar_buf[:], in_=barsb[:])
            bar_ci = nc.gpsimd.collective_compute(
                "AllReduce",
                mybir.AluOpType.add,
                replica_groups=[list(range(N_CORES))],
                ins=[bar_buf[:]],
                outs=[bar_out[:]],
            )
            barsb2 = cp.tile([1, 8], F32, tag="barsb2")
            bar_dma = nc.sync.dma_start(out=barsb2[:], in_=bar_out[:])


            # r_0 partial
            r0sel = lp.tile([B, NS], F16, tag="rselt")
            nc.gpsimd.dma_start(out=r0sel[:], in_=rsel[0])
            prod0 = lp.tile([B, NS], F16, tag="prod")
            nc.vector.tensor_tensor(
                out=prod0[:], in0=x0r_sb[:], in1=r0sel[:],
                op=mybir.AluOpType.mult,
            )
            nc.vector.tensor_reduce(
                out=racc[:, 0:1], in_=prod0[:],
                axis=mybir.AxisListType.X, op=mybir.AluOpType.add,
            )

            first_bcasts = []
            n_iter = l_steps - 1
            for t in range(n_iter):
                mstep = lp.tile([128, A * 256], F16, tag="mstep")
                nc.gpsimd.dma_start(out=mstep[:], in_=masks[t])
                rselt = lp.tile([B, NS], F16, tag="rselt")
                nc.gpsimd.dma_start(out=rselt[:], in_=rsel[t + 1])

                # masked X^T halves; [128, 8, 32] views pick each half's
                # 32-col region out of every rank's 64-col block
                xha = lp.tile([128, A * 256], F16, tag="xha")
                xhb = lp.tile([128, A * 256], F16, tag="xhb")
                if t == 0:
                    xt3 = x0_sb[:].rearrange("p (r q) -> p r q", q=64)
                else:
                    base = 512 * ((t - 1) % 2)
                    xt3 = xt_raw[:, base:base + 512].rearrange(
                        "p (r q) -> p r q", q=64)
                first_xh = {}
                for h, xh in ((0, xha), (1, xhb)):
                    for a in range(A):
                        inst = nc.vector.tensor_tensor(
                            out=xh[:, a * 256:(a + 1) * 256].rearrange(
                                "p (r q) -> p r q", q=32),
                            in0=xt3[:, :, 32 * h:32 * h + 32],
                            in1=mstep[:, a * 256:(a + 1) * 256].rearrange(
                                "p (r q) -> p r q", q=32),
                            op=mybir.AluOpType.mult,
                        )
                        if a == 0:
                            first_xh[h] = inst
                if t > 0:
                    par = (t - 1) % 2
                    thr = 16 * ((t - 1) // 2 + 1)
                    wait_targets.append((first_xh[0].ins, sems[par][0].num, thr))
                    wait_targets.append((first_xh[1].ins, sems[par][1].num, thr))

                pmA = ppa.tile([128, HC], F32, tag="pmA")
                pmB = ppb.tile([128, HC], F32, tag="pmB")
                last = t == n_iter - 1
                pp_ = t % 2

                bounces = []
                stages = []
                for half, pm in ((0, pmA), (1, pmB)):
                    off = half * HC
                    for i, kt in enumerate(kt_order):
                        xh = xha if kt % NT < 2 else xhb
                        for a in range(A):
                            nc.tensor.matmul(
                                out=pm[32 * a:32 * a + 8, :],
                                lhsT=lhs_slice(xh, a, kt),
                                rhs=t_sb[:, (a * KT + kt) * NS + off:
                                         (a * KT + kt) * NS + off + HC],
                                start=(i == 0),
                                stop=(i == KT - 1),
                                tile_position=(0, 32 * a),
                            )
                    # fold 4 col groups into rows 0..7 of a 16-row tile
                    c0 = lp.tile([B, HC], F32, tag=f"c0{half}")
                    nc.scalar.activation(
                        out=c0[:], in_=pm[0:8, :],
                        func=mybir.ActivationFunctionType.Copy,
                    )
                    c1 = lp.tile([B, HC], F32, tag=f"c1{half}")
                    nc.vector.tensor_tensor(
                        out=c1[:], in0=c0[:], in1=pm[32:40, :],
                        op=mybir.AluOpType.add,
                    )
                    c2 = lp.tile([B, HC], F32, tag=f"c2{half}")
                    nc.vector.tensor_tensor(
                        out=c2[:], in0=c1[:], in1=pm[64:72, :],
                        op=mybir.AluOpType.add,
                    )
                    b16 = lp.tile([16, HC], F16, tag=f"b16{half}")
                    nc.vector.tensor_tensor(
                        out=b16[0:8, :], in0=c2[:], in1=pm[96:104, :],
                        op=mybir.AluOpType.add,
                    )
                    bounces.append(b16)

                if not last:
                    par = t % 2
                    for half, b16 in ((0, bounces[0]), (1, bounces[1])):
                        stage = lp.tile([128, 32], F16, tag=f"stage{half}")
                        nc.sync.dma_start(
                            out=stage[:].rearrange("p (di m) -> p di m", di=2),
                            in_=b16[:].rearrange("m (di do) -> m di do", do=128),
                            transpose=True,
                        )
                        # slot s of every core holds the block from rank
                        # (self ^ s): one single-dest XOR-relative send per
                        # slot, so every AP is compile-time static.
                        for s_ in range(N_CORES):
                            col = 512 * par + 64 * s_ + 32 * half
                            bc = nc.gpsimd.remote_dma_broadcast(
                                out_ap=xt_raw[:, col:col + 32],
                                in_ap=stage[:],
                                remote_sem=sems[par][half],
                                local_sem=lsem,
                                rdests=[(0, s_) if j == s_ else None
                                        for j in range(N_CORES)],
                            )
                            if t == 0:
                                first_bcasts.append(bc)
                        nc.gpsimd.trigger_dma(count=N_CORES)

                # reward partials r_{t+1}
                prod = lp.tile([B, NS], F16, tag="prod")
                nc.vector.tensor_tensor(
                    out=prod[:, 0:HC], in0=bounces[0][0:8, :], in1=rselt[:, 0:HC],
                    op=mybir.AluOpType.mult,
                )
                nc.vector.tensor_tensor(
                    out=prod[:, HC:NS], in0=bounces[1][0:8, :], in1=rselt[:, HC:NS],
                    op=mybir.AluOpType.mult,
                )
                nc.vector.tensor_reduce(
                    out=racc[:, t + 1:t + 2], in_=prod[:],
                    axis=mybir.AxisListType.X, op=mybir.AluOpType.add,
                )

            # final: AllReduce reward partials, store
            nc.sync.dma_start(out=ar_in[:], in_=racc[:])
            nc.gpsimd.collective_compute(
                "AllReduce",
                mybir.AluOpType.add,
                replica_groups=[list(range(N_CORES))],
                ins=[ar_in[:]],
                outs=[ar_out[:]],
            )
            rfin = cp.tile([B, l_steps], F32, tag="rfin")
            nc.sync.dma_start(out=rfin[:], in_=ar_out[:])
            nc.gpsimd.dma_start(out=out[:], in_=rfin[:])

            for bc in first_bcasts:
                tile.add_dep_helper(
                    bc.ins, bar_dma.ins,
                    reason="no P2P send before the all-core startup barrier",
                )

    _inject_sem_waits(nc, mybir, wait_targets)
    _split_waits(nc, mybir)
    return nc


def _inject_sem_waits(nc, mybir, targets):
    """Insert a NoOp carrying `sem >= thr` immediately before each target
    instruction on its engine (post-scheduling, like _split_waits)."""
    by_inst = {id(inst): (sem_num, thr) for inst, sem_num, thr in targets}
    for bb in nc.main_func.blocks:
        insts = list(bb.instructions)
        new = []
        changed = False
        for ins in insts:
            hit = by_inst.get(id(ins))
            if hit is not None:
                sem_num, thr = hit
                new.append(
                    mybir.InstNoOp(
                        name=f"{ins.name}-xwait",
                        sync_info=mybir.SyncInfo(
                            on_wait=[mybir.SyncWait(
                                sync_type="semaphore",
                                id=sem_num,
                                wait_mode="sem-ge-imm",
                                wait_value=thr,
                            )],
                            on_update=[],
                        ),
                        bass_nofuse=True,
                        engine=ins.engine,
                    )
                )
                changed = True
            new.append(ins)
        if changed:
            live = bb.instructions
            live[:] = new


PROBE_PHASES = [
    (0,), (0, 1), (0, 1, 2), (0, 1, 2, 3),
    (0, 2), (1, 3), (1, 2, 3), (0, 1, 2, 3),
]



def _build_v4(l_steps: int, n_junk: int = 0):
    """Single AllGather per step (the collective engine serializes calls, so
    two half-gathers cost ~15us/step), rewards on the DVE, and junk matmuls
    spanning the exchange gap so the sweep runs at the warm 2.4 GHz clock."""
    from concourse import bass, tile
    from concourse.bass import mybir

    F32 = mybir.dt.float32
    F16 = mybir.dt.float16

    nc = bass.Bass(num_devices=N_CORES)

    t_tiles = nc.declare_dram_parameter("t_tiles", [128, A * KT * NS], F16, isOutput=False)
    x0t = nc.declare_dram_parameter("x0t", [128, NT * 64], F16, isOutput=False)
    x0row = nc.declare_dram_parameter("x0row", [B, NS], F16, isOutput=False)
    masks = nc.declare_dram_parameter("masks", [max(l_steps - 1, 1), 128, A * NT * 64], F16, isOutput=False)
    rsel = nc.declare_dram_parameter("rsel", [l_steps, B, NS], F16, isOutput=False)
    out = nc.declare_dram_parameter("out", [B, l_steps], F32, isOutput=True)

    cc_in = [nc.dram_tensor(f"cc_in{i}", [B, NS], F16) for i in range(2)]
    cc_out = [
        nc.dram_tensor(f"cc_out{i}", [N_CORES * B, NS], F16, addr_space="Shared")
        for i in range(2)
    ]
    ar_in = nc.dram_tensor("ar_in", [B, l_steps], F32)
    ar_out = nc.dram_tensor("ar_out", [B, l_steps], F32, addr_space="Shared")

    with tile.TileContext(nc) as tc:
        with tc.tile_pool(name="const", bufs=1) as cp, \
             tc.tile_pool(name="loop", bufs=3) as lp, \
             tc.tile_pool(name="ps", bufs=2, space="PSUM") as pmp, \
             tc.tile_pool(name="psj", bufs=1, space="PSUM") as pjp:

            t_sb = cp.tile([128, A * KT * NS], F16, tag="t_sb")
            nc.sync.dma_start(out=t_sb[:], in_=t_tiles[:])
            x0_sb = cp.tile([128, NT * 64], F16, tag="x0_sb")
            nc.sync.dma_start(out=x0_sb[:], in_=x0t[:])
            x0r_sb = cp.tile([B, NS], F16, tag="x0r_sb")
            nc.sync.dma_start(out=x0r_sb[:], in_=x0row[:])
            racc = cp.tile([B, l_steps], F32, tag="racc")

            r0sel = lp.tile([B, NS], F16, tag="rselt")
            nc.gpsimd.dma_start(out=r0sel[:], in_=rsel[0])
            prod0 = lp.tile([B, NS], F16, tag="prod")
            nc.vector.tensor_tensor(
                out=prod0[:], in0=x0r_sb[:], in1=r0sel[:],
                op=mybir.AluOpType.mult,
            )
            nc.vector.tensor_reduce(
                out=racc[:, 0:1], in_=prod0[:],
                axis=mybir.AxisListType.X, op=mybir.AluOpType.add,
            )

            xt_prev = x0_sb
            n_iter = l_steps - 1
            for t in range(n_iter):
                mstep = lp.tile([128, A * NT * 64], F16, tag="mstep")
                nc.gpsimd.dma_start(out=mstep[:], in_=masks[t])
                rselt = lp.tile([B, NS], F16, tag="rselt")
                nc.gpsimd.dma_start(out=rselt[:], in_=rsel[t + 1])

                xhat = lp.tile([128, A * NT * 64], F16, tag="xhat")
                for a in range(A):
                    nc.vector.tensor_tensor(
                        out=xhat[:, a * 256:(a + 1) * 256],
                        in0=xt_prev[:, 0:256],
                        in1=mstep[:, a * 256:(a + 1) * 256],
                        op=mybir.AluOpType.mult,
                    )

                pm = pmp.tile([128, NS], F32, tag="pm")
                for kt in range(KT):
                    for a in range(A):
                        r_, nt_ = kt // NT, kt % NT
                        lhsT = xhat[:, a * 256 + nt_ * 64 + r_ * 8:
                                    a * 256 + nt_ * 64 + r_ * 8 + 8]
                        nc.tensor.matmul(
                            out=pm[32 * a:32 * a + 8, :],
                            lhsT=lhsT,
                            rhs=t_sb[:, (a * KT + kt) * NS:(a * KT + kt + 1) * NS],
                            start=(kt == 0),
                            stop=(kt == KT - 1),
                            tile_position=(0, 32 * a),
                        )

                last = t == n_iter - 1

                # HAM warmth across the exchange gap
                if n_junk > 0 and not last:
                    pj = pjp.tile([128, NS], F32, tag="pj")
                    for j in range(n_junk):
                        nc.tensor.matmul(
                            out=pj[0:8, :],
                            lhsT=xhat[:, 0:8],
                            rhs=t_sb[:, (j % 128) * NS:(j % 128 + 1) * NS],
                            start=True, stop=True,
                            tile_position=(0, 0),
                            skip_group_check=True,
                        )

                # fold: ACT evacuates group 0, DVE chains the 3 adds
                c0 = lp.tile([B, NS], F32, tag="c0")
                nc.scalar.activation(
                    out=c0[:], in_=pm[0:8, :],
                    func=mybir.ActivationFunctionType.Copy,
                )
                c1 = lp.tile([B, NS], F32, tag="c1")
                nc.vector.tensor_tensor(
                    out=c1[:], in0=c0[:], in1=pm[32:40, :],
                    op=mybir.AluOpType.add,
                )
                c2 = lp.tile([B, NS], F32, tag="c2")
                nc.vector.tensor_tensor(
                    out=c2[:], in0=c1[:], in1=pm[64:72, :],
                    op=mybir.AluOpType.add,
                )
                bounce = lp.tile([B, NS], F16, tag="bounce")
                nc.vector.tensor_tensor(
                    out=bounce[:], in0=c2[:], in1=pm[96:104, :],
                    op=mybir.AluOpType.add,
                )

                # reward partials r_{t+1}
                prod = lp.tile([B, NS], F16, tag="prod")
                nc.vector.tensor_tensor(
                    out=prod[:], in0=bounce[:], in1=rselt[:],
                    op=mybir.AluOpType.mult,
                )
                nc.vector.tensor_reduce(
                    out=racc[:, t + 1:t + 2], in_=prod[:],
                    axis=mybir.AxisListType.X, op=mybir.AluOpType.add,
                )

                if last:
                    break

                pp = t % 2
                nc.sync.dma_start(out=cc_in[pp][:], in_=bounce[:])
                nc.gpsimd.collective_compute(
                    "AllGather",
                    mybir.AluOpType.bypass,
                    replica_groups=[list(range(N_CORES))],
                    ins=[cc_in[pp][:]],
                    outs=[cc_out[pp][:]],
                )
                xt = lp.tile([128, NT * 64], F16, tag="xt")
                nc.sync.dma_start(
                    out=xt[:].rearrange("p (di m) -> p di m", di=NT),
                    in_=cc_out[pp][:].rearrange("m (di do) -> m di do", do=128),
                    transpose=True,
                )
                xt_prev = xt

            nc.sync.dma_start(out=ar_in[:], in_=racc[:])
            nc.gpsimd.collective_compute(
                "AllReduce",
                mybir.AluOpType.add,
                replica_groups=[list(range(N_CORES))],
                ins=[ar_in[:]],
                outs=[ar_out[:]],
            )
            rfin = cp.tile([B, l_steps], F32, tag="rfin")
            nc.sync.dma_start(out=rfin[:], in_=ar_out[:])
            nc.gpsimd.dma_start(out=out[:], in_=rfin[:])

    _split_waits(nc, mybir)
    return nc


def _build_probe2(n_iter: int = 48):
    """Isolate what breaks 4-way col-group concurrency in the real sweep."""
    from concourse import bass, tile
    from concourse.bass import mybir

    F32 = mybir.dt.float32
    F16 = mybir.dt.float16

    nc = bass.Bass(num_devices=N_CORES)
    t_tiles = nc.declare_dram_parameter("t_tiles", [128, A * KT * NS], F16, isOutput=False)
    x0t = nc.declare_dram_parameter("x0t", [128, A * NT * 64], F16, isOutput=False)
    out = nc.declare_dram_parameter("out", [B, 8], F32, isOutput=True)

    CH = 8  # kts per accumulation chain

    with tile.TileContext(nc) as tc:
        with tc.tile_pool(name="const", bufs=1) as cp, \
             tc.tile_pool(name="ps", bufs=2, space="PSUM") as pp, \
             tc.tile_pool(name="psr", bufs=1, space="PSUM") as prp:
            t_sb = cp.tile([128, A * KT * NS], F16, tag="t_sb")
            nc.sync.dma_start(out=t_sb[:], in_=t_tiles[:])
            x_sb = cp.tile([128, A * NT * 64], F16, tag="x_sb")
            nc.sync.dma_start(out=x_sb[:], in_=x0t[:])
            marker = cp.tile([128, 512], F16, tag="marker")
            psum_rew = prp.tile([128, 512], F32, tag="rew")

            def lhs(a, kt):
                r_, nt_ = kt // NT, kt % NT
                c = a * 256 + nt_ * 64 + r_ * 8
                return x_sb[:, c:c + 8]

            # phase a: independent MMs, near rhs slices (probe-1 repro)
            pm = pp.tile([128, NS], F32, tag="pm")
            for it in range(n_iter * CH):
                for g in range(A):
                    nc.tensor.matmul(
                        out=pm[32 * g:32 * g + 8, :],
                        lhsT=lhs(g, it % 4),
                        rhs=t_sb[:, ((it + g) % 4) * NS:((it + g) % 4) * NS + NS],
                        start=True, stop=True,
                        tile_position=(0, 32 * g),
                        skip_group_check=True,
                    )
            nc.vector.memset(marker[:], 1.0)

            # phase b: 8-kt accumulation chains, near rhs slices
            for it in range(n_iter):
                pm = pp.tile([128, NS], F32, tag="pm")
                for kt in range(CH):
                    for g in range(A):
                        nc.tensor.matmul(
                            out=pm[32 * g:32 * g + 8, :],
                            lhsT=lhs(g, kt % 4),
                            rhs=t_sb[:, ((kt + g) % 4) * NS:((kt + g) % 4) * NS + NS],
                            start=(kt == 0), stop=(kt == CH - 1),
                            tile_position=(0, 32 * g),
                        )
            nc.vector.memset(marker[:], 2.0)

            # phase c: chains + REAL far-apart rhs offsets
            for it in range(n_iter):
                pm = pp.tile([128, NS], F32, tag="pm")
                for kt in range(CH):
                    for g in range(A):
                        nc.tensor.matmul(
                            out=pm[32 * g:32 * g + 8, :],
                            lhsT=lhs(g, kt),
                            rhs=t_sb[:, (g * KT + kt) * NS:(g * KT + kt) * NS + NS],
                            start=(kt == 0), stop=(kt == CH - 1),
                            tile_position=(0, 32 * g),
                        )
            nc.vector.memset(marker[:], 3.0)

            # phase d: phase c + reward MMs appended per chain
            for it in range(n_iter):
                pm = pp.tile([128, NS], F32, tag="pm")
                for kt in range(CH):
                    for g in range(A):
                        nc.tensor.matmul(
                            out=pm[32 * g:32 * g + 8, :],
                            lhsT=lhs(g, kt),
                            rhs=t_sb[:, (g * KT + kt) * NS:(g * KT + kt) * NS + NS],
                            start=(kt == 0), stop=(kt == CH - 1),
                            tile_position=(0, 32 * g),
                        )
                for kt in range(CH):
                    for g in range(A):
                        nc.tensor.matmul(
                            out=psum_rew[32 * g:32 * g + 8, it % 512:it % 512 + 1],
                            lhsT=lhs(g, kt),
                            rhs=t_sb[:, (g * KT + kt) * NS:(g * KT + kt) * NS + 1],
                            start=(kt == 0), stop=(kt == CH - 1),
                            tile_position=(0, 32 * g),
                            skip_group_check=True,
                        )
            nc.vector.memset(marker[:], 4.0)

            res = cp.tile([B, 8], F32, tag="res")
            nc.vector.tensor_copy(out=res[:], in_=pm[0:8, 0:8])
            nc.gpsimd.dma_start(out=out[:], in_=res[:])

    _split_waits(nc, mybir)
    return nc


def _build_probe(n_iter: int = 200):
    """Microbench: measure PE column-group concurrency per PROBE_PHASES."""
    from concourse import bass, tile
    from concourse.bass import mybir

    F32 = mybir.dt.float32
    F16 = mybir.dt.float16

    nc = bass.Bass(num_devices=N_CORES)
    t_tiles = nc.declare_dram_parameter("t_tiles", [128, 4 * 512], F16, isOutput=False)
    x0t = nc.declare_dram_parameter("x0t", [128, 64], F16, isOutput=False)
    out = nc.declare_dram_parameter("out", [B, 8], F32, isOutput=True)

    with tile.TileContext(nc) as tc:
        with tc.tile_pool(name="const", bufs=1) as cp, \
             tc.tile_pool(name="ps", bufs=2, space="PSUM") as pp:
            t_sb = cp.tile([128, 4 * 512], F16, tag="t_sb")
            nc.sync.dma_start(out=t_sb[:], in_=t_tiles[:])
            x_sb = cp.tile([128, 64], F16, tag="x_sb")
            nc.sync.dma_start(out=x_sb[:], in_=x0t[:])
            marker = cp.tile([128, 512], F16, tag="marker")

            for phase, groups in enumerate(PROBE_PHASES):
                pm = pp.tile([128, 512], F32, tag="pm")
                for it in range(n_iter):
                    for g in groups:
                        nc.tensor.matmul(
                            out=pm[32 * g:32 * g + 8, :],
                            lhsT=x_sb[:, 8 * g:8 * g + 8],
                            rhs=t_sb[:, 512 * ((it + g) % 4):512 * ((it + g) % 4) + 512],
                            start=True, stop=True,
                            tile_position=(0, 32 * g),
                            skip_group_check=True,
                        )
                # phase marker: big DVE op (visible in trace) + PSUM drain
                nc.vector.memset(marker[:], float(phase))

            res = cp.tile([B, 8], F32, tag="res")
            nc.vector.tensor_copy(out=res[:], in_=pm[0:8, 0:8])
            nc.gpsimd.dma_start(out=out[:], in_=res[:])

    _split_waits(nc, mybir)
    return nc


def _split_waits(nc, mybir, max_waits: int = 1):
    """Walrus rejects >1 sem wait on DMA/CTRL structs; spill extras to NoOps."""
    for bb in nc.main_func.blocks:
        insts = list(bb.instructions)
        new = []
        changed = False
        for ins in insts:
            si = getattr(ins, "sync_info", None)
            if si is not None and len(si.on_wait) > max_waits:
                waits = list(si.on_wait)
                for k, w in enumerate(waits[:-max_waits]):
                    new.append(
                        mybir.InstNoOp(
                            name=f"{ins.name}-wsplit{k}",
                            sync_info=mybir.SyncInfo(on_wait=[w], on_update=[]),
                            bass_nofuse=True,
                            engine=ins.engine,
                        )
                    )
                ins.sync_info = mybir.SyncInfo(
                    on_wait=waits[-max_waits:], on_update=list(si.on_update)
                )
                changed = True
            new.append(ins)
        if changed:
            live = bb.instructions
            live[:] = new


def _prepare_inputs(init_states, actions, T, R, l_steps, variant="full"):
    init_states = np.asarray(init_states).astype(np.int64)
    actions = np.asarray(actions).astype(np.int64)
    T = np.asarray(T, dtype=np.float32)
    R = np.asarray(R, dtype=np.float32)

    T16 = T.astype(np.float16)
    R16 = R.astype(np.float16)

    if variant in ("v2", "v4"):
        # x0row[b, :] = X0[b, r*NS : (r+1)*NS] per core r (one-hot rows)
        # rsel[t, b, :] = R16[actions[b, t], r*NS : (r+1)*NS]
        # masks as in the baseline, truncated to l_steps-1 transitions
        x0t = np.zeros((128, NT, 64), dtype=np.float16)
        for b in range(B):
            s0 = int(init_states[b])
            r_, rem = divmod(s0, NS)
            nt_, p = divmod(rem, 128)
            x0t[p, nt_, r_ * 8 + b] = 1.0
        x0t = x0t.reshape(128, NT * 64)

        n_tr = max(l_steps - 1, 1)
        onehot = (actions.T[:, None, :] == np.arange(A)[None, :, None])  # [L, A, B]
        masks = np.broadcast_to(
            onehot[:, None, :, None, None, :], (actions.shape[1], 128, A, NT, N_CORES, B)
        ).astype(np.float16).reshape(actions.shape[1], 128, A * NT * 64)
        masks = np.ascontiguousarray(masks[:n_tr])

        rsel_full = R16[actions[:, :l_steps]]          # [B, l_steps, S]
        in_maps = []
        for r_ in range(N_CORES):
            tc_ = T16[:, :, r_ * NS:(r_ + 1) * NS]
            tt = np.ascontiguousarray(
                tc_.reshape(A, KT, 128, NS).transpose(2, 0, 1, 3).reshape(128, A * KT * NS)
            )
            x0row = np.zeros((B, NS), dtype=np.float16)
            for b in range(B):
                s0 = int(init_states[b])
                if r_ * NS <= s0 < (r_ + 1) * NS:
                    x0row[b, s0 - r_ * NS] = 1.0
            rsel_c = np.ascontiguousarray(
                rsel_full[:, :, r_ * NS:(r_ + 1) * NS].transpose(1, 0, 2)
            )  # [l_steps, B, NS]
            in_maps.append({
                "t_tiles": tt,
                "x0t": x0t,
                "x0row": x0row,
                "masks": masks,
                "rsel": rsel_c,
            })
        return in_maps

    if variant == "v3":
        # xt col = 64*slot + 32*(nt//2) + 16*(nt%2) + b  (b 8..15 junk);
        # slot s on core d holds the block from rank d^s, so the k-tile
        # kt = s*4+nt of core d's T tile must read input rows of rank d^s.
        n_tr = max(l_steps - 1, 1)
        onehot = (actions.T[:, None, :] == np.arange(A)[None, :, None])  # [L, A, B]
        m6 = np.zeros((actions.shape[1], A, N_CORES, 2, 16), np.float16)
        m6[:, :, :, :, :8] = onehot[:, :, None, None, :]
        masks = np.broadcast_to(
            m6[:, None], (actions.shape[1], 128, A, N_CORES, 2, 16)
        ).reshape(actions.shape[1], 128, A * 256)
        masks = np.ascontiguousarray(masks[:n_tr])

        rsel_full = R16[actions[:, :l_steps]]          # [B, l_steps, S]
        in_maps = []
        for d in range(N_CORES):
            tc_ = T16[:, :, d * NS:(d + 1) * NS]       # [A, S, NS]
            # tt[p, ((a*8 + s)*4 + nt)*NS + n] = T[a, (d^s)*512 + nt*128 + p, n]
            blocks = np.empty((A, N_CORES, NT, 128, NS), np.float16)
            for s in range(N_CORES):
                src = d ^ s
                blocks[:, s] = tc_[:, src * NS:(src + 1) * NS, :].reshape(
                    A, NT, 128, NS)
            tt = np.ascontiguousarray(
                blocks.transpose(3, 0, 1, 2, 4).reshape(128, A * KT * NS))

            x0t = np.zeros((128, N_CORES, 2, 2, 16), dtype=np.float16)
            for b in range(B):
                s0 = int(init_states[b])
                src, rem = divmod(s0, NS)
                nt_, p = divmod(rem, 128)
                x0t[p, d ^ src, nt_ // 2, nt_ % 2, b] = 1.0
            x0t = x0t.reshape(128, N_CORES * 64)

            x0row = np.zeros((B, NS), dtype=np.float16)
            for b in range(B):
                s0 = int(init_states[b])
                if d * NS <= s0 < (d + 1) * NS:
                    x0row[b, s0 - d * NS] = 1.0
            rsel_c = np.ascontiguousarray(
                rsel_full[:, :, d * NS:(d + 1) * NS].transpose(1, 0, 2)
            )
            in_maps.append({
                "t_tiles": tt,
                "x0t": x0t,
                "x0row": x0row,
                "masks": masks,
                "rsel": rsel_c,
            })
        return in_maps

    if variant == "probe":
        tt = np.ascontiguousarray(T16[0, :128 * 1, :].reshape(128, -1)[:, :4 * 512])
        x0t = np.zeros((128, 64), np.float16)
        x0t[:, :] = 0.01
        return [{"t_tiles": tt, "x0t": x0t} for _ in range(N_CORES)]

    if variant == "probe2":
        tc_ = T16[:, :, 0:NS]
        tt = np.ascontiguousarray(
            tc_.reshape(A, KT, 128, NS).transpose(2, 0, 1, 3).reshape(128, A * KT * NS)
        )
        x0t = np.full((128, A * NT * 64), 0.01, np.float16)
        return [{"t_tiles": tt, "x0t": x0t} for _ in range(N_CORES)]

    # r_tiles[p, a*KT + kt] = R[a, kt*128 + p]
    r_tiles = np.ascontiguousarray(
        R16.reshape(A, KT, 128).transpose(2, 0, 1).reshape(128, A * KT)
    )

    # x0t[p, nt*64 + r*8 + b] = X0[b, r*512 + nt*128 + p]
    x0t = np.zeros((128, NT, 64), dtype=np.float16)
    for b in range(B):
        s0 = int(init_states[b])
        r_, rem = divmod(s0, NS)
        nt_, p = divmod(rem, 128)
        x0t[p, nt_, r_ * 8 + b] = 1.0
    x0t = x0t.reshape(128, NT * 64)

    # masks[t, p, a*256 + nt*64 + r*8 + b] = (actions[b, t] == a), all p
    lfull = actions.shape[1]
    onehot = (actions.T[:, None, :] == np.arange(A)[None, :, None])  # [L, A, B]
    masks = np.broadcast_to(
        onehot[:, None, :, None, None, :], (lfull, 128, A, NT, N_CORES, B)
    ).astype(np.float16).reshape(lfull, 128, A * NT * 64)
    masks = np.ascontiguousarray(masks[:l_steps])

    # per-core T tiles: t_tiles[p, (a*KT + kt)*NS + n] = T[a, kt*128+p, r*NS+n]
    in_maps = []
    for r_ in range(N_CORES):
        tc_ = T16[:, :, r_ * NS:(r_ + 1) * NS]                 # [A, S, NS]
        tt = np.ascontiguousarray(
            tc_.reshape(A, KT, 128, NS).transpose(2, 0, 1, 3).reshape(128, A * KT * NS)
        )
        in_maps.append({
            "t_tiles": tt,
            "r_tiles": r_tiles,
            "x0t": x0t,
            "masks": masks,
        })
    return in_maps


def _run(init_states, actions, T, R, l_steps=L, trace=False):
    from concourse.bass_utils import run_bass_kernel_spmd

    import os as _os
    variant = _os.environ.get("KVARIANT", "full")
    n_repeat = int(_os.environ.get("KREPEAT", "1"))
    n_junk = int(_os.environ.get("KJUNK", "0"))
    key = (l_steps, variant, n_repeat, n_junk)
    if key not in _CACHE:
        if variant == "split2":
            _CACHE[key] = _build_split2(l_steps, n_repeat)
        elif variant == "v2":
            _CACHE[key] = _build_v2(l_steps, n_junk)
        elif variant == "v4":
            _CACHE[key] = _build_v4(l_steps, n_junk)
        elif variant == "probe":
            _CACHE[key] = _build_probe()
        elif variant == "probe2":
            _CACHE[key] = _build_probe2()
        elif variant == "v3":
            _CACHE[key] = _build_v3(l_steps)
        else:
            _CACHE[key] = _build(l_steps, variant, n_repeat, n_junk)
    nc = _CACHE[key]
    in_maps = _prepare_inputs(init_states, actions, T, R, l_steps, variant)
    res = run_bass_kernel_spmd(
        nc, in_maps, list(range(N_CORES)), trace=trace
    )
    rewards = res.results[0]["out"].astype(np.float32)
    return rewards, res


def kernel(init_states, actions, T, R):
    rewards, _ = _run(init_states, actions, T, R, l_steps=L, trace=False)
    return rewards

